# revision 1
# baseline (speedup 1.0000x reference)
# Trainium2 Bass kernel for nn_DecoderAttention (gnn_message_passing).
# Self-contained: host-side prep (numpy) + bass/Tile device kernel + SPMD run.
#
# Sharding: 8 cores = batch(2) x H-quarters(4). Each core handles one batch
# element and 24 output rows (+1 halo row each side), all 4 contexts.
# On-chip layout: channels on partitions (context pairs stacked to 128),
# pixels row-major in the free dim with row pitch 98 (1 zero col between
# rows serves as both left/right conv padding).
import numpy as np
import ml_dtypes

E, D, K, B, H, W = 64, 128, 4, 2, 96, 96
RP = 98
ROWS_OUT = 24
ROWS_IN = ROWS_OUT + 2
NBUF = 2552               # padded in-buffer width (26*98=2548, +4 slack, even)
OB = RP + 1               # out-window base = 99
NW = ROWS_OUT * RP - 2    # 2350
CH = 470
NCH = 5
TIN = 2352                # tanh-input tile width (NW+2, even)
NCORES = 8

BF = ml_dtypes.bfloat16
DELTAS = [(r, c) for r in range(3) for c in range(3)]
DOFF = [(r - 1) * RP + (c - 1) for r, c in DELTAS]

# in-buffer chunking for keys (NBUF-wide psum stages)
KCH = [(i * 512, min(512, NBUF - i * 512)) for i in range(5)]

_CACHE = {}


def _bf16(x):
    return np.ascontiguousarray(np.asarray(x, np.float32).astype(BF))


def _f32(x):
    return np.ascontiguousarray(np.asarray(x, np.float32))


# ---------------------------------------------------------------- host prep
def _pad_to_buf(img, r0):
    """img [C, H, W] -> [C, NBUF] zero-padded halo'd row-pitch-98 buffer."""
    C = img.shape[0]
    out = np.zeros((C, NBUF), np.float32)
    lr = np.arange(ROWS_IN)
    gr = r0 - 1 + lr
    ok = (gr >= 0) & (gr < H)
    for i in np.nonzero(ok)[0]:
        out[:, i * RP + 1: i * RP + 1 + W] = img[:, gr[i], :]
    return out


def _prep_weights(inputs):
    w_enc, b_enc = _f32(inputs["w_enc"]), _f32(inputs["b_enc"])
    w_dec, b_dec = _f32(inputs["w_dec"]), _f32(inputs["b_dec"])
    w_agg = _f32(inputs["w_agg"])
    w_val, b_val = _f32(inputs["w_val"]), _f32(inputs["b_val"])
    w_attn, b_attn = _f32(inputs["w_attn"]), _f32(inputs["b_attn"])

    Wdec_dup = np.empty((128, 128), np.float32)
    for m in range(128):
        Wdec_dup[:, m] = w_dec[m % 64, :]
    WencB = np.zeros((128, 128), np.float32)
    WencB[:64, :64] = w_enc.T
    WencB[64:, 64:] = w_enc.T
    Wval = np.zeros((128, 9, 64), np.float32)
    for di, (r, c) in enumerate(DELTAS):
        Wval[:, di, :] = w_val[:, :, r, c].T
    Wagg32 = np.zeros((128, 32), np.float32)
    for i in range(32):
        j = i % 2
        Wagg32[j * 64:(j + 1) * 64, i] = w_agg
    Wzrep = np.zeros((96, 96), np.float32)
    rr = np.arange(96)
    for m in range(96):
        Wzrep[(rr % 2) == (m % 2), m] = 1.0 / 16.0
    # B-broadcast lhsT: K=32 block (identical for each cg block): rows i,
    # cols m: 1/16 where i%2 == m//64
    Wbc32 = np.zeros((96, 128), np.float32)
    for i in range(96):
        Wbc32[i, :] = 0.0
        half = (i % 2) * 64
        Wbc32[i, 64 - half:128 - half] = 0.0  # no-op, clarity
    ii = np.arange(96) % 2
    mm_ = np.arange(128) // 64
    Wbc32[:, :] = (ii[:, None] == mm_[None, :]) / 16.0
    WattnP = np.zeros((128, 2, 64), np.float32)
    for p in range(2):
        for pp in range(128):
            kk = 2 * p + pp // 64
            WattnP[pp, p, :] = w_attn[:, 64 * (1 + kk) + (pp % 64)]
    Wattn_v = np.ascontiguousarray(w_attn[:, :64].T)
    I96 = np.eye(96, dtype=np.float32)
    b_qk = np.tile(b_dec + b_enc, 2).reshape(128, 1)
    b_out = (w_attn[:, :64] @ b_val + b_attn).reshape(64, 1)

    return dict(
        Wdec_dup=_bf16(Wdec_dup), WencB=_bf16(WencB), Wval=_bf16(Wval),
        Wagg32=_bf16(Wagg32), Wzrep=_bf16(Wzrep), Wbc3=_bf16(Wbc32),
        WattnP=_bf16(WattnP), Wattn_v=_bf16(Wattn_v), I96=_bf16(I96),
        b_qk=_f32(b_qk), b_out=_f32(b_out),
    )


def _prep_mask(r0):
    """mask96 [96, NCH*3*CH] bf16; rows (cg,i) replicated; delta = 3*cg+slot."""
    px = np.arange(NW)
    oh, w = px // RP, px % RP
    m = np.zeros((96, NCH, 3, CH), np.float32)
    for di, (r, c) in enumerate(DELTAS):
        gr = r0 + oh + (r - 1)
        gc = w + (c - 1)
        ok = (gr >= 0) & (gr < H) & (gc >= 0) & (gc < W)
        # pad columns (w>=96): never output, but keep Z nonzero to avoid
        # inf/NaN from the reciprocal
        ok = ok | (w >= 96)
        cg, s = di // 3, di % 3
        m[cg * 32:(cg + 1) * 32, :, s, :] = ok.astype(np.float32).reshape(
            NCH, CH)[None]
    return _bf16(m.reshape(96, NCH * 3 * CH))


def host_prep(inputs):
    wt = _prep_weights(inputs)
    ctx = _f32(inputs["contexts"])
    dec = _f32(inputs["decoded_features"])
    cores = []
    for b in range(B):
        for q in range(4):
            r0 = q * ROWS_OUT
            ctx01 = np.zeros((128, NBUF), np.float32)
            ctx01[:64] = _pad_to_buf(ctx[0, b], r0)
            ctx01[64:] = _pad_to_buf(ctx[1, b], r0)
            ctx23 = np.zeros((128, NBUF), np.float32)
            ctx23[:64] = _pad_to_buf(ctx[2, b], r0)
            ctx23[64:] = _pad_to_buf(ctx[3, b], r0)
            core = dict(
                dec=_bf16(_pad_to_buf(dec[b], r0)),
                ctx01=_bf16(ctx01), ctx23=_bf16(ctx23),
                mask96=_prep_mask(r0),
            )
            core.update(wt)
            cores.append(core)
    return cores


# ---------------------------------------------------------------- bass build
def build_nc():
    import concourse.bacc as bacc
    import concourse.bass as bass
    import concourse.mybir as mybir
    import concourse.tile as tile

    f32d = mybir.dt.float32
    bf16d = mybir.dt.bfloat16
    AF = mybir.ActivationFunctionType
    OP = mybir.AluOpType

    nc = bacc.Bacc("TRN2", target_bir_lowering=False, debug=False,
                   num_devices=NCORES)

    # DRAM tensors
    d_dec = nc.dram_tensor("dec", [128, NBUF], bf16d, kind="ExternalInput").ap()
    d_ctx = [nc.dram_tensor("ctx01", [128, NBUF], bf16d, kind="ExternalInput").ap(),
             nc.dram_tensor("ctx23", [128, NBUF], bf16d, kind="ExternalInput").ap()]
    d_mask = nc.dram_tensor("mask96", [96, NCH * 3 * CH], bf16d,
                            kind="ExternalInput").ap()
    d_Wdec = nc.dram_tensor("Wdec_dup", [128, 128], bf16d, kind="ExternalInput").ap()
    d_Wenc = nc.dram_tensor("WencB", [128, 128], bf16d, kind="ExternalInput").ap()
    d_Wval = nc.dram_tensor("Wval", [128, 9, 64], bf16d, kind="ExternalInput").ap()
    d_Wagg = nc.dram_tensor("Wagg32", [128, 32], bf16d, kind="ExternalInput").ap()
    d_Wz = nc.dram_tensor("Wzrep", [96, 96], bf16d, kind="ExternalInput").ap()
    d_Wbc = nc.dram_tensor("Wbc3", [96, 128], bf16d, kind="ExternalInput").ap()
    d_WaP = nc.dram_tensor("WattnP", [128, 2, 64], bf16d, kind="ExternalInput").ap()
    d_Wav = nc.dram_tensor("Wattn_v", [64, 64], bf16d, kind="ExternalInput").ap()
    d_I96 = nc.dram_tensor("I96", [96, 96], bf16d, kind="ExternalInput").ap()
    d_bqk = nc.dram_tensor("b_qk", [128, 1], f32d, kind="ExternalInput").ap()
    d_bout = nc.dram_tensor("b_out", [64, 1], f32d, kind="ExternalInput").ap()
    d_out = nc.dram_tensor("out", [64, ROWS_OUT, 96], f32d,
                           kind="ExternalOutput").ap()
    import os
    dbg = os.environ.get("BASSDBG") == "1"
    abl = os.environ.get("ABL", "")
    d_dbg = {}
    if dbg:
        for nm, sh, dt in [("dbg_q", [128, TIN], bf16d),
                           ("dbg_keys0", [128, NBUF], bf16d),
                           ("dbg_values", [64, NW], bf16d),
                           ("dbg_exp0", [96, NCH * 3 * CH], bf16d),
                           ("dbg_rz0", [96, NW], bf16d),
                           ("dbg_P00", [128, NW], bf16d),
                           ("dbg_tanh00", [128, TIN], bf16d),
                           ("dbg_acc", [64, NW], f32d)]:
            d_dbg[nm] = nc.dram_tensor(nm, sh, dt, kind="ExternalOutput").ap()

    with tile.TileContext(nc) as tc:
        from contextlib import ExitStack
        stack = ExitStack()
        with tc.tile_pool(name="const", bufs=1) as const, \
             tc.tile_pool(name="big", bufs=1) as big, \
             tc.tile_pool(name="work", bufs=2) as work, \
             tc.tile_pool(name="tanhp", bufs=10) as tanhp, \
             tc.tile_pool(name="pp", bufs=3) as pp:
            psA = stack.enter_context(
                tc.tile_pool(name="psA", bufs=2, space="PSUM"))
            psS = stack.enter_context(
                tc.tile_pool(name="psS", bufs=2, space="PSUM"))

            # ---- load constants/inputs
            def load(pool, ap, shape, dtype, tag):
                t = pool.tile(shape, dtype, tag=tag, name=tag)
                nc.sync.dma_start(out=t, in_=ap)
                return t

            Wdec = load(const, d_Wdec, [128, 128], bf16d, "Wdec")
            Wenc = load(const, d_Wenc, [128, 128], bf16d, "Wenc")
            Wval = load(const, d_Wval, [128, 9, 64], bf16d, "Wval")
            Wagg = load(const, d_Wagg, [128, 32], bf16d, "Wagg")
            Wz = load(const, d_Wz, [96, 96], bf16d, "Wz")
            Wbc = load(const, d_Wbc, [96, 128], bf16d, "Wbc")
            WaP = load(const, d_WaP, [128, 2, 64], bf16d, "WaP")
            Wav = load(const, d_Wav, [64, 64], bf16d, "Wav")
            I96 = load(const, d_I96, [96, 96], bf16d, "I96")
            bqk = load(const, d_bqk, [128, 1], f32d, "bqk")
            bout = load(const, d_bout, [64, 1], f32d, "bout")
            mask_s = load(const, d_mask, [96, NCH * 3 * CH], bf16d, "mask")
            dec_s = load(big, d_dec, [128, NBUF], bf16d, "dec")
            ctx_s = [load(big, d_ctx[0], [128, NBUF], bf16d, "ctx01"),
                     load(big, d_ctx[1], [128, NBUF], bf16d, "ctx23")]

            # ---- S2: queries -> q_dup [128, TIN] bf16 (+ b_qk), q_odd
            q_dup = big.tile([128, TIN], bf16d, tag="q_dup", name="q_dup")
            nc.vector.memset(q_dup[:, NW:TIN], 0.0)
            for c in range(NCH):
                psq = psA.tile([128, 512], f32d, tag="mm", name="psq")
                nc.tensor.matmul(psq[:, 0:CH], Wdec,
                                 dec_s[:, OB + c * CH: OB + (c + 1) * CH],
                                 start=True, stop=True)
                nc.scalar.activation(out=q_dup[:, c * CH:(c + 1) * CH],
                                     in_=psq[:, 0:CH], func=AF.Identity,
                                     bias=bqk, scale=1.0)
            q_odd = big.tile([128, TIN], bf16d, tag="q_odd", name="q_odd")
            nc.vector.memset(q_odd[:, 0:2], 0.0)
            nc.vector.memset(q_odd[:, TIN - 2:TIN], 0.0)
            nc.vector.tensor_copy(q_odd[:, 1:NW + 1], q_dup[:, 0:NW])
            if dbg:
                nc.sync.dma_start(out=d_dbg["dbg_q"], in_=q_dup)

            # ---- S4: values conv -> values_s [64, NW] bf16 (M-split 2-way)
            values_s = big.tile([64, NW], bf16d, tag="values", name="values_s")
            for c in range(NCH):
                psv = psA.tile([64, 512], f32d, tag="mm", name="psv")
                for di in range(9):
                    base = OB + DOFF[di] + c * CH
                    for h in range(2):
                        nc.tensor.matmul(
                            psv[32 * h:32 * (h + 1), 0:CH],
                            Wval[:, di, 32 * h:32 * (h + 1)],
                            dec_s[:, base: base + CH],
                            start=(di == 0), stop=(di == 8),
                            tile_position=(0, 32 * h))
                nc.vector.tensor_copy(values_s[:, c * CH:(c + 1) * CH],
                                      psv[:, 0:CH])
            if dbg:
                nc.sync.dma_start(out=d_dbg["dbg_values"], in_=values_s)

            exp_tiles = []
            for p in range(2):
                # ---- S3: keys
                keys_s = big.tile([128, NBUF], bf16d, tag="keys", bufs=2,
                                  name=f"keys{p}")
                for (off, wn) in KCH:
                    psk = psA.tile([128, 512], f32d, tag="mm", name="psk")
                    nc.tensor.matmul(psk[:, 0:wn], Wenc,
                                     ctx_s[p][:, off:off + wn],
                                     start=True, stop=True)
                    nc.vector.tensor_copy(keys_s[:, off:off + wn],
                                          psk[:, 0:wn])
                if dbg and p == 0:
                    nc.sync.dma_start(out=d_dbg["dbg_keys0"], in_=keys_s)

                # ---- S5/S6: tanh tiles
                tanh_t = []
                for di in range(9):
                    base = OB + DOFF[di]
                    tin = work.tile([128, TIN], bf16d, tag="tin", bufs=3,
                                    name="tin")
                    if base % 2 == 1:
                        nc.vector.tensor_add(tin, q_odd,
                                             keys_s[:, base - 1: base - 1 + TIN])
                        off = 1
                    else:
                        nc.vector.tensor_add(tin, q_dup,
                                             keys_s[:, base: base + TIN])
                        off = 0
                    th = tanhp.tile([128, TIN], bf16d, tag="tanh",
                                    name=f"tanh{p}_{di}")
                    nc.scalar.activation(out=th, in_=tin, func=AF.Tanh)
                    tanh_t.append((th, off))
                    if dbg and p == 0 and di == 0:
                        nc.sync.dma_start(out=d_dbg["dbg_tanh00"], in_=th)

                # ---- score pipeline per chunk
                exp_s = big.tile([96, NCH, 3, CH], bf16d, tag="exp_s", bufs=2,
                                 name=f"exp_s{p}")
                if abl == "noscore":
                    nc.vector.memset(exp_s, 0.1)
                    exp_tiles.append(exp_s)
                    continue
                rz96 = big.tile([96, NW], bf16d, tag="rz", bufs=2,
                                name=f"rz{p}")
                for c in range(NCH):
                    ps_s = psS.tile([128, 3, 512], f32d, tag="s", name="ps_s")
                    for di in range(9):
                        cg, s = di // 3, di % 3
                        th, off = tanh_t[di]
                        nc.tensor.matmul(
                            ps_s[32 * cg:32 * (cg + 1), s, 0:CH], Wagg,
                            th[:, off + c * CH: off + c * CH + CH],
                            start=True, stop=True,
                            tile_position=(0, 32 * cg))
                    nc.scalar.activation(out=exp_s[:, c, :, :],
                                         in_=ps_s[0:96, :, 0:CH], func=AF.Exp)
                    # mask
                    nc.vector.tensor_mul(
                        exp_s[:, c, :, :], exp_s[:, c, :, :],
                        mask_s[:, c * 3 * CH:(c + 1) * 3 * CH].rearrange(
                            "p (s f) -> p s f", f=CH))
                    # Z (pre-broadcast, replicated rows) + reciprocal
                    psz = psA.tile([96, 512], f32d, tag="mm", name="psz")
                    for s in range(3):
                        nc.tensor.matmul(psz[:, 0:CH], Wz,
                                         exp_s[:, c, s, :],
                                         start=(s == 0), stop=(s == 2))
                    with nc.allow_low_precision(
                            reason="softmax probs tolerate bf16 recip"):
                        nc.vector.reciprocal(out=rz96[:, c * CH:(c + 1) * CH],
                                             in_=psz[:, 0:CH])
                    # normalize (rz broadcast over the slot dim via step-0 AP)
                    rzv = rz96[:, c * CH:(c + 1) * CH]
                    rzb = bass.AP(tensor=rzv.tensor, offset=rzv.offset,
                                  ap=[list(rzv.ap[0]), [0, 3], list(rzv.ap[1])])
                    nc.vector.tensor_mul(exp_s[:, c, :, :],
                                         exp_s[:, c, :, :], rzb)

                exp_tiles.append(exp_s)
                if dbg and p == 0:
                    nc.sync.dma_start(
                        out=d_dbg["dbg_exp0"].rearrange(
                            "p (c s f) -> p c s f", s=3, f=CH),
                        in_=exp_s)
                    nc.sync.dma_start(out=d_dbg["dbg_rz0"], in_=rz96)

            # score-phase PSUM freed; open the output accumulators
            stack.close()
            psO = stack.enter_context(
                tc.tile_pool(name="psO", bufs=5, space="PSUM"))
            psB = stack.enter_context(
                tc.tile_pool(name="psB", bufs=3, space="PSUM"))
            pso = [psO.tile([64, 512], f32d, tag="o", name=f"pso{c}")
                   for c in range(NCH)]
            for c in range(NCH):
                nc.tensor.matmul(pso[c][:, 0:CH], Wav,
                                 values_s[:, c * CH:(c + 1) * CH],
                                 start=True, stop=(abl == "noprod"))

            # ---- B broadcast + products P + final accumulation
            # delta order interleaves cg0 so consecutive K=32 B-matmuls hit
            # different PE row groups and run concurrently.
            DORD = [0, 3, 6, 1, 4, 7, 2, 5, 8]
            if abl == "noprod":
                DORD = []
            # product path: odd-base deltas must use the stt path (bf16 2x
            # mul needs 4B alignment); N_ACT of the rest go ACT-evac + mul.
            N_ACT = 10
            act_assigned = 0
            for p in range(2):
                exp_s = exp_tiles[p]
                for di in DORD:
                    cg0, s0 = di // 3, di % 3
                    base = OB + DOFF[di]
                    odd = (base % 2 == 1)  # NOTE: base odd => aligned even!
                    # base parity: even base -> bf16-aligned operand
                    use_act = (base % 2 == 0) and act_assigned < N_ACT
                    if use_act:
                        act_assigned += 1
                    P = pp.tile([128, NW], bf16d, tag="P", name=f"P{p}_{di}")
                    for c in range(NCH):
                        psb = psB.tile([128, 512], f32d, tag="b", name="psb")
                        nc.tensor.matmul(
                            psb[:, 0:CH],
                            Wbc[32 * cg0:32 * (cg0 + 1), :],
                            exp_s[32 * cg0:32 * (cg0 + 1), c, s0, :],
                            start=True, stop=True,
                            tile_position=(32 * cg0, 0))
                        if use_act:
                            bev = work.tile([128, CH], bf16d, tag="bev",
                                            bufs=6, name="bev")
                            nc.scalar.copy(out=bev, in_=psb[:, 0:CH])
                            nc.vector.tensor_mul(
                                P[:, c * CH:(c + 1) * CH], bev,
                                ctx_s[p][:, base + c * CH: base + c * CH + CH])
                        else:
                            nc.vector.scalar_tensor_tensor(
                                out=P[:, c * CH:(c + 1) * CH],
                                in0=psb[:, 0:CH], scalar=1.0,
                                in1=ctx_s[p][:, base + c * CH: base + c * CH + CH],
                                op0=OP.mult, op1=OP.mult)
                    for c in range(NCH):
                        nc.tensor.matmul(
                            pso[c][:, 0:CH], WaP[:, p, :],
                            P[:, c * CH:(c + 1) * CH],
                            start=False, stop=(p == 1 and di == DORD[-1]))
                    if dbg and p == 0 and di == 0:
                        nc.sync.dma_start(out=d_dbg["dbg_P00"], in_=P)

            # ---- LeakyReLU + DMA out: t = psum + b_out; out = max(t, 0.2*t)
            out_s = big.tile([64, TIN], f32d, tag="out_s", name="out_s")
            acc_s = big.tile([64, NW], f32d, tag="acc_s", name="acc_s")
            nc.vector.memset(out_s[:, NW:TIN], 0.0)
            for c in range(NCH):
                nc.scalar.activation(out=acc_s[:, c * CH:(c + 1) * CH],
                                     in_=pso[c][:, 0:CH], func=AF.Identity,
                                     bias=bout, scale=1.0)
                nc.vector.scalar_tensor_tensor(
                    out=out_s[:, c * CH:(c + 1) * CH],
                    in0=acc_s[:, c * CH:(c + 1) * CH], scalar=0.2,
                    in1=acc_s[:, c * CH:(c + 1) * CH],
                    op0=OP.mult, op1=OP.max)
            if dbg:
                nc.sync.dma_start(out=d_dbg["dbg_acc"], in_=acc_s)
            nc.sync.dma_start(
                out=d_out,
                in_=out_s.rearrange("p (h rp) -> p h rp", rp=RP)[:, :, 0:96])
            stack.close()

    nc.compile()
    return nc


def _get_nc():
    if "nc" not in _CACHE:
        _CACHE["nc"] = build_nc()
    return _CACHE["nc"]


# ---------------------------------------------------------------- entry point
def kernel(**inputs):
    from concourse import bass_utils

    cores = host_prep(inputs)
    nc = _get_nc()
    res = bass_utils.run_bass_kernel_spmd(
        nc, [dict(c) for c in cores], core_ids=list(range(NCORES)))
    out = np.zeros((B, E, H, W), np.float32)
    for b in range(B):
        for q in range(4):
            o = res.results[b * 4 + q]["out"]
            out[b, :, q * ROWS_OUT:(q + 1) * ROWS_OUT, :] = o
    return out



# revision 4
# speedup vs baseline: 1.5824x; 1.5824x over previous
# Trainium2 Bass kernel for nn_DecoderAttention (gnn_message_passing), v2.
# Self-contained: host-side prep (numpy) + bass/Tile device kernel + SPMD run.
#
# Sharding: 8 cores = batch(2) x H-quarters(4). Each core: one batch element,
# 24 output rows (+1 halo row each side), all 4 contexts.
#
# v2 design (vs v1): scores are computed into PIXEL-major layout via
# [128px, 2]-output matmuls (cheap under the cost model: matmul cost is the
# output free size), the whole softmax (exp/mask/Z/recip/normalize) runs in
# pixel-major where probabilities are per-partition scalars, then p is
# PE-transposed back to channel-major rows and broadcast to 64 channels with
# K=2 selector matmuls. w_attn is folded into the values conv and into
# per-(delta,pair) fold matmuls so everything accumulates in one PSUM tile
# per output pixel-tile. Output is written pixel-major bf16.
import numpy as np
import ml_dtypes

E, D, K, B, H, W = 64, 128, 4, 2, 96, 96
RP = 98
ROWS_OUT = 24
ROWS_IN = ROWS_OUT + 2
NBUF = 2656               # in-buffer width (26*98=2548, padded for +-99 shifts)
OB = RP + 1               # out-window base = 99
NW = ROWS_OUT * RP - 2    # 2350 real out-window pixels
T = 19                    # pixel tiles of 128 (19*128 = 2432 >= NW)
PW = T * 128              # 2432
TIN = PW + 2              # tanh tile width (even)
NCORES = 8

BF = ml_dtypes.bfloat16
DELTAS = [(r, c) for r in range(3) for c in range(3)]
DOFF = [(r - 1) * RP + (c - 1) for r, c in DELTAS]

# chunk grids
QCH = [(i * 512, min(512, TIN - i * 512)) for i in range(5)]      # q over TIN
KCH = [(i * 512, min(512, NBUF - i * 512)) for i in range(6)]     # keys
PCH = [(i * 512, min(512, PW - i * 512)) for i in range(5)]       # products

# product stt engine split: per (pair, delta) group -> 'dve' | 'pool'
# (set after measuring; pool requires gpsimd PSUM reads to work)
PROD_ENGINE = ["dve"] * 18

_CACHE = {}


def _bf16(x):
    return np.ascontiguousarray(np.asarray(x, np.float32).astype(BF))


def _f32(x):
    return np.ascontiguousarray(np.asarray(x, np.float32))


# ---------------------------------------------------------------- host prep
def _pad_to_buf(img, r0):
    """img [C, H, W] -> [C, NBUF] zero-padded halo'd row-pitch-98 buffer."""
    C = img.shape[0]
    out = np.zeros((C, NBUF), np.float32)
    lr = np.arange(ROWS_IN)
    gr = r0 - 1 + lr
    ok = (gr >= 0) & (gr < H)
    for i in np.nonzero(ok)[0]:
        out[:, i * RP + 1: i * RP + 1 + W] = img[:, gr[i], :]
    return out


def _prep_weights(inputs):
    w_enc, b_enc = _f32(inputs["w_enc"]), _f32(inputs["b_enc"])
    w_dec, b_dec = _f32(inputs["w_dec"]), _f32(inputs["b_dec"])
    w_agg = _f32(inputs["w_agg"])
    w_val, b_val = _f32(inputs["w_val"]), _f32(inputs["b_val"])
    w_attn, b_attn = _f32(inputs["w_attn"]), _f32(inputs["b_attn"])

    Wdec_dup = np.empty((128, 128), np.float32)
    for m in range(128):
        Wdec_dup[:, m] = w_dec[m % 64, :]
    WencB = np.zeros((128, 128), np.float32)
    WencB[:64, :64] = w_enc.T
    WencB[64:, 64:] = w_enc.T
    Wagg2 = np.zeros((128, 2), np.float32)
    Wagg2[:64, 0] = w_agg
    Wagg2[64:, 1] = w_agg
    I128 = np.eye(128, dtype=np.float32)
    # per-delta broadcast selectors (per pair layout): K=18 rows d*2+k
    Sel18 = np.zeros((18, 9, 128), np.float32)
    for di in range(9):
        Sel18[di * 2, di, :64] = 1.0
        Sel18[di * 2 + 1, di, 64:] = 1.0
    # w_attn folded into the 3x3 values conv: [d, delta, o]
    WvalT = np.zeros((128, 9, 64), np.float32)
    for di, (r, c) in enumerate(DELTAS):
        # v'[o](x) = sum_e w_attn[o, e] * w_val[e, d, r, c] * dec[d](x+delta)
        WvalT[:, di, :] = (w_attn[:, :64] @ w_val[:, :, r, c]).T
    # fold matmuls rhs per pair: [(k*64+e), o] = w_attn[o, 64*(1+2p+k)+e]
    WfoldP = np.zeros((128, 2, 64), np.float32)
    for p in range(2):
        for k in range(2):
            blk = w_attn[:, 64 * (1 + 2 * p + k): 64 * (2 + 2 * p + k)]
            WfoldP[64 * k: 64 * (k + 1), p, :] = blk.T
    ones1 = np.ones((1, 128), np.float32)
    b_out = (w_attn[:, :64] @ b_val + b_attn).reshape(1, 64)
    b_qk = np.tile(b_dec + b_enc, 2).reshape(128, 1)

    return dict(
        Wdec_dup=_bf16(Wdec_dup), WencB=_bf16(WencB), Wagg2=_bf16(Wagg2),
        I128=_bf16(I128), Sel18=_bf16(Sel18), WvalT=_bf16(WvalT),
        WfoldP=_bf16(WfoldP), ones1=_bf16(ones1), brow=_bf16(b_out),
        b_qk=_f32(b_qk),
    )


def _prep_mask(r0):
    """mask_pix [128, T*36] bf16: per pixel tile t, partition P, col di*4+k."""
    s = np.arange(T * 128)
    pos = OB + s
    lr = pos // RP - 1
    c = pos % RP - 1
    valid_px = (s < NW) & (c >= 0) & (c < W) & (lr >= 0) & (lr < ROWS_OUT)
    m = np.zeros((T * 128, 9), np.float32)
    for di, (r, cc) in enumerate(DELTAS):
        dr, dc = r - 1, cc - 1
        ok = valid_px & (r0 + lr + dr >= 0) & (r0 + lr + dr < H) \
            & (c + dc >= 0) & (c + dc < W)
        m[:, di] = ok.astype(np.float32)
    # junk pixels: keep center neighbor on so Z > 0 (no inf/NaN downstream)
    m[~valid_px, 4] = 1.0
    m2 = np.repeat(m[:, :, None], 2, axis=2)          # [T*128, 9, 2]
    m2 = m2.reshape(T, 128, 18).transpose(1, 0, 2)    # [128, T, 18]
    return _bf16(m2.reshape(128, T * 18))


def host_prep(inputs):
    wt = _prep_weights(inputs)
    ctx = _f32(inputs["contexts"])
    dec = _f32(inputs["decoded_features"])
    cores = []
    for b in range(B):
        for q in range(4):
            r0 = q * ROWS_OUT
            ctx01 = np.zeros((128, NBUF), np.float32)
            ctx01[:64] = _pad_to_buf(ctx[0, b], r0)
            ctx01[64:] = _pad_to_buf(ctx[1, b], r0)
            ctx23 = np.zeros((128, NBUF), np.float32)
            ctx23[:64] = _pad_to_buf(ctx[2, b], r0)
            ctx23[64:] = _pad_to_buf(ctx[3, b], r0)
            core = dict(
                dec=_bf16(_pad_to_buf(dec[b], r0)),
                ctx01=_bf16(ctx01), ctx23=_bf16(ctx23),
                mask_pix=_prep_mask(r0),
            )
            core.update(wt)
            cores.append(core)
    return cores


# ---------------------------------------------------------------- bass build
def build_nc():
    import concourse.bacc as bacc
    import concourse.bass as bass
    import concourse.mybir as mybir
    import concourse.tile as tile

    f32d = mybir.dt.float32
    bf16d = mybir.dt.bfloat16
    AF = mybir.ActivationFunctionType
    OP = mybir.AluOpType

    nc = bacc.Bacc("TRN2", target_bir_lowering=False, debug=False,
                   num_devices=NCORES)

    d_dec = nc.dram_tensor("dec", [128, NBUF], bf16d, kind="ExternalInput").ap()
    d_ctx = [nc.dram_tensor("ctx01", [128, NBUF], bf16d, kind="ExternalInput").ap(),
             nc.dram_tensor("ctx23", [128, NBUF], bf16d, kind="ExternalInput").ap()]
    d_mask = nc.dram_tensor("mask_pix", [128, T * 36], bf16d,
                            kind="ExternalInput").ap()
    d_Wdec = nc.dram_tensor("Wdec_dup", [128, 128], bf16d, kind="ExternalInput").ap()
    d_Wenc = nc.dram_tensor("WencB", [128, 128], bf16d, kind="ExternalInput").ap()
    d_Wagg = nc.dram_tensor("Wagg2", [128, 2], bf16d, kind="ExternalInput").ap()
    d_I128 = nc.dram_tensor("I128", [128, 128], bf16d, kind="ExternalInput").ap()
    d_Sel = nc.dram_tensor("SelAll", [36, 18 * 128], bf16d,
                           kind="ExternalInput").ap()
    d_WvalT = nc.dram_tensor("WvalT", [128, 9, 64], bf16d, kind="ExternalInput").ap()
    d_WfoldP = nc.dram_tensor("WfoldP", [128, 2, 64], bf16d,
                              kind="ExternalInput").ap()
    d_ones1 = nc.dram_tensor("ones1", [1, 128], bf16d, kind="ExternalInput").ap()
    d_brow = nc.dram_tensor("brow", [1, 64], bf16d, kind="ExternalInput").ap()
    d_bqk = nc.dram_tensor("b_qk", [128, 1], f32d, kind="ExternalInput").ap()
    d_out = nc.dram_tensor("out", [128, T * 64], bf16d, kind="ExternalOutput").ap()

    with tile.TileContext(nc) as tc:
        from contextlib import ExitStack
        stack = ExitStack()
        with tc.tile_pool(name="const", bufs=1) as const, \
             tc.tile_pool(name="big", bufs=1) as big, \
             tc.tile_pool(name="keysp", bufs=2) as keysp, \
             tc.tile_pool(name="work", bufs=3) as work, \
             tc.tile_pool(name="tanhp", bufs=6) as tanhp, \
             tc.tile_pool(name="pp", bufs=2) as pp, \
             tc.tile_pool(name="psO", bufs=1, space="PSUM") as psO:
            psS = stack.enter_context(
                tc.tile_pool(name="psS", bufs=1, space="PSUM"))
            psA = stack.enter_context(
                tc.tile_pool(name="psA", bufs=2, space="PSUM"))

            def load(pool, ap, shape, dtype, tag):
                t = pool.tile(shape, dtype, tag=tag, name=tag)
                nc.sync.dma_start(out=t, in_=ap)
                return t

            Wdec = load(const, d_Wdec, [128, 128], bf16d, "Wdec")
            Wenc = load(const, d_Wenc, [128, 128], bf16d, "Wenc")
            Wagg = load(const, d_Wagg, [128, 2], bf16d, "Wagg")
            I128 = load(const, d_I128, [128, 128], bf16d, "I128")
            SelAll = load(const, d_Sel, [36, 18, 128], bf16d, "SelAll")
            WvalT = load(const, d_WvalT, [128, 9, 64], bf16d, "WvalT")
            WfoldP = load(const, d_WfoldP, [128, 2, 64], bf16d, "WfoldP")
            ones1 = load(const, d_ones1, [1, 128], bf16d, "ones1")
            brow = load(const, d_brow, [1, 64], bf16d, "brow")
            bqk = load(const, d_bqk, [128, 1], f32d, "bqk")
            mask_s = load(const, d_mask, [128, T, 36], bf16d, "mask")
            dec_s = load(big, d_dec, [128, NBUF], bf16d, "dec")
            ctx_s = [load(big, d_ctx[0], [128, NBUF], bf16d, "ctx01"),
                     load(big, d_ctx[1], [128, NBUF], bf16d, "ctx23")]

            # ---- output accumulator psum: [128px, T*64] f32
            psO_all = psO.tile([128, T * 64], f32d, tag="o", name="psO_all")

            # ---- values conv (w_attn folded), opens every psO region
            for di in range(9):
                base = OB + DOFF[di]
                for t in range(T):
                    nc.tensor.matmul(
                        psO_all[:, t * 64:(t + 1) * 64],
                        dec_s[:, base + t * 128: base + (t + 1) * 128],
                        WvalT[:, di, :],
                        start=(di == 0), stop=False)

            # ---- queries -> q_dup [128, TIN] bf16 (+ b_qk), q_odd
            q_dup = big.tile([128, TIN], bf16d, tag="q_dup", name="q_dup")
            nc.vector.memset(q_dup[:, PW:TIN], 0.0)
            for (c0, cw) in QCH:
                psq = psA.tile([128, 512], f32d, tag="mm", name="psq")
                nc.tensor.matmul(psq[:, 0:cw], Wdec,
                                 dec_s[:, OB + c0: OB + c0 + cw],
                                 start=True, stop=True)
                nc.scalar.activation(out=q_dup[:, c0:c0 + cw],
                                     in_=psq[:, 0:cw], func=AF.Identity,
                                     bias=bqk, scale=1.0)
            q_odd = big.tile([128, TIN], bf16d, tag="q_odd", name="q_odd")
            nc.gpsimd.memset(q_odd[:, 0:2], 0.0)
            nc.gpsimd.tensor_copy(q_odd[:, 1:TIN], q_dup[:, 0:TIN - 1])

            # ---- scores psum [128, T, 9, 4] + per-pair keys/tanh
            psS_all = psS.tile([128, T, 9, 4], f32d, tag="s", name="psS_all")
            for p in range(2):
                keys_s = keysp.tile([128, NBUF], bf16d, tag="keys",
                                    name=f"keys{p}")
                for (c0, cw) in KCH:
                    psk = psA.tile([128, 512], f32d, tag="mm", name="psk")
                    nc.tensor.matmul(psk[:, 0:cw], Wenc,
                                     ctx_s[p][:, c0:c0 + cw],
                                     start=True, stop=True)
                    nc.vector.tensor_copy(keys_s[:, c0:c0 + cw], psk[:, 0:cw])
                for di in range(9):
                    base = OB + DOFF[di]
                    tin = work.tile([128, TIN], bf16d, tag="tin", name="tin")
                    # offload 2 of 9 adds per pair to the (otherwise idle)
                    # gpsimd engine
                    eng_a = nc.gpsimd if di in (0, 5) else nc.vector
                    if base % 2 == 1:
                        eng_a.tensor_add(tin, q_odd,
                                         keys_s[:, base - 1: base - 1 + TIN])
                        off = 1
                    else:
                        eng_a.tensor_add(tin, q_dup,
                                         keys_s[:, base: base + TIN])
                        off = 0
                    th = tanhp.tile([128, TIN], bf16d, tag="tanh",
                                    name=f"tanh{p}_{di}")
                    nc.scalar.activation(out=th, in_=tin, func=AF.Tanh)
                    for t in range(T):
                        nc.tensor.matmul(
                            psS_all[:, t, di, 2 * p: 2 * p + 2],
                            th[:, off + t * 128: off + t * 128 + 128],
                            Wagg, start=True, stop=True)

            # ---- softmax in pixel-major
            p_pix = big.tile([128, T, 36], bf16d, tag="p_pix", name="p_pix")
            Zbuf = big.tile([128, T, 4], f32d, tag="Z", name="Zbuf")
            rz = big.tile([128, T, 4], bf16d, tag="rz", name="rz")
            for t in range(T):
                nc.scalar.activation(
                    out=p_pix[:, t, :],
                    in_=psS_all[:, t].rearrange("p a b -> p (a b)"),
                    func=AF.Exp)
                nc.vector.tensor_mul(p_pix[:, t, :], p_pix[:, t, :],
                                     mask_s[:, t, :])
                nc.vector.tensor_reduce(
                    out=Zbuf[:, t, :],
                    in_=p_pix[:, t, :].rearrange("p (d k) -> p k d", k=4),
                    axis=mybir.AxisListType.X, op=OP.add)
            with nc.allow_low_precision(reason="softmax recip in bf16"):
                nc.vector.reciprocal(
                    out=rz.rearrange("p a b -> p (a b)"),
                    in_=Zbuf.rearrange("p a b -> p (a b)"))
            for t in range(T):
                rzv = rz[:, t, :]
                rzb = bass.AP(tensor=rzv.tensor, offset=rzv.offset,
                              ap=[list(rzv.ap[0]), [0, 9], list(rzv.ap[-1])])
                nc.vector.tensor_mul(
                    p_pix[:, t, :].rearrange("p (d k) -> p d k", k=4),
                    p_pix[:, t, :].rearrange("p (d k) -> p d k", k=4), rzb)

            # ---- transpose p to channel-major rows [36, PW]
            pT_sb = big.tile([36, PW], bf16d, tag="pT", name="pT_sb")
            stack.close()   # free psA; keep psO/psS... (psS dead now too)
            psT = stack.enter_context(
                tc.tile_pool(name="psT", bufs=2, space="PSUM"))
            psB = stack.enter_context(
                tc.tile_pool(name="psB", bufs=3, space="PSUM"))
            for batch in range((T + 3) // 4):
                n_in = min(4, T - batch * 4)
                pst = psT.tile([36, 512], bf16d, tag="t", name="psT")
                for tt in range(n_in):
                    t = batch * 4 + tt
                    nc.tensor.transpose(pst[:, tt * 128:(tt + 1) * 128],
                                        p_pix[:, t, :], I128)
                nc.vector.tensor_copy(
                    pT_sb[:, batch * 512: batch * 512 + n_in * 128],
                    pst[:, 0:n_in * 128])

            # ---- products + folds per (pair, delta)
            gi = 0
            for p in range(2):
                for di in range(9):
                    base = OB + DOFF[di]
                    g = p * 9 + di
                    prods = pp.tile([128, PW], bf16d, tag="prods",
                                    name=f"prods{p}_{di}")
                    eng = nc.vector if PROD_ENGINE[gi] == "dve" else nc.gpsimd
                    gi += 1
                    for (c0, cw) in PCH:
                        psb = psX.tile([128, 512], f32d, tag="x", name="psb")
                        nc.tensor.matmul(psb[:, 0:cw], SelAll[:, g, :],
                                         pT_sb[:, c0:c0 + cw],
                                         start=True, stop=True)
                        eng.scalar_tensor_tensor(
                            out=prods[:, c0:c0 + cw],
                            in0=psb[:, 0:cw], scalar=1.0,
                            in1=ctx_s[p][:, base + c0: base + c0 + cw],
                            op0=OP.mult, op1=OP.mult)
                    for t in range(T):
                        nc.tensor.matmul(
                            psO_all[:, t * 64:(t + 1) * 64],
                            prods[:, t * 128:(t + 1) * 128],
                            WfoldP[:, p, :], start=False, stop=False)

            # ---- bias + leaky + out
            for t in range(T):
                nc.tensor.matmul(psO_all[:, t * 64:(t + 1) * 64],
                                 ones1, brow, start=False, stop=True)
            o_f = big.tile([128, T * 64], bf16d, tag="o_f", name="o_f")
            nc.scalar.activation(out=o_f, in_=psO_all, func=AF.Identity)
            o_sb = big.tile([128, T * 64], bf16d, tag="o_sb", name="o_sb")
            nc.vector.scalar_tensor_tensor(
                out=o_sb, in0=o_f, scalar=0.2, in1=o_f,
                op0=OP.mult, op1=OP.max)
            nc.sync.dma_start(out=d_out, in_=o_sb)
            stack.close()

    nc.compile()
    return nc


def _get_nc():
    if "nc" not in _CACHE:
        _CACHE["nc"] = build_nc()
    return _CACHE["nc"]


# ---------------------------------------------------------------- entry point
def _assemble(results):
    out = np.zeros((B, E, H, W), np.float32)
    s = np.arange(NW)
    pos = OB + s
    lr = pos // RP - 1
    c = pos % RP - 1
    sel = (c >= 0) & (c < W)
    for b in range(B):
        for q in range(4):
            res = np.asarray(results[b * 4 + q]["out"]).astype(np.float32)
            res = res.reshape(128, T, 64).transpose(1, 0, 2).reshape(T * 128, 64)
            out[b, :, q * ROWS_OUT + lr[sel], c[sel]] = res[s[sel]]
    return out


def kernel(**inputs):
    from concourse import bass_utils

    cores = host_prep(inputs)
    nc = _get_nc()
    res = bass_utils.run_bass_kernel_spmd(
        nc, [dict(c) for c in cores], core_ids=list(range(NCORES)))
    return _assemble(res.results)


# revision 5
# speedup vs baseline: 1.6996x; 1.0741x over previous
# Trainium2 Bass kernel for nn_DecoderAttention (gnn_message_passing), v2.
# Self-contained: host-side prep (numpy) + bass/Tile device kernel + SPMD run.
#
# Sharding: 8 cores = batch(2) x H-quarters(4). Each core: one batch element,
# 24 output rows (+1 halo row each side), all 4 contexts.
#
# v2 design (vs v1): scores are computed into PIXEL-major layout via
# [128px, 2]-output matmuls (cheap under the cost model: matmul cost is the
# output free size), the whole softmax (exp/mask/Z/recip/normalize) runs in
# pixel-major where probabilities are per-partition scalars, then p is
# PE-transposed back to channel-major rows and broadcast to 64 channels with
# K=2 selector matmuls. w_attn is folded into the values conv and into
# per-(delta,pair) fold matmuls so everything accumulates in one PSUM tile
# per output pixel-tile. Output is written pixel-major bf16.
import numpy as np
import ml_dtypes

E, D, K, B, H, W = 64, 128, 4, 2, 96, 96
RP = 98
ROWS_OUT = 24
ROWS_IN = ROWS_OUT + 2
NBUF = 2656               # in-buffer width (26*98=2548, padded for +-99 shifts)
OB = RP + 1               # out-window base = 99
NW = ROWS_OUT * RP - 2    # 2350 real out-window pixels
T = 19                    # pixel tiles of 128 (19*128 = 2432 >= NW)
PW = T * 128              # 2432
TIN = PW + 2              # tanh tile width (even)
NCORES = 8

BF = ml_dtypes.bfloat16
DELTAS = [(r, c) for r in range(3) for c in range(3)]
DOFF = [(r - 1) * RP + (c - 1) for r, c in DELTAS]

# chunk grids
QCH = [(i * 512, min(512, TIN - i * 512)) for i in range(5)]      # q over TIN
KCH = [(i * 512, min(512, NBUF - i * 512)) for i in range(6)]     # keys
PCH = [(i * 512, min(512, PW - i * 512)) for i in range(5)]       # products

# product stt engine split: per (pair, delta) group -> 'dve' | 'pool'
# (set after measuring; pool requires gpsimd PSUM reads to work)
PROD_ENGINE = ["dve"] * 18

_CACHE = {}


def _bf16(x):
    return np.ascontiguousarray(np.asarray(x, np.float32).astype(BF))


def _f32(x):
    return np.ascontiguousarray(np.asarray(x, np.float32))


# ---------------------------------------------------------------- host prep
def _pad_to_buf(img, r0):
    """img [C, H, W] -> [C, NBUF] zero-padded halo'd row-pitch-98 buffer."""
    C = img.shape[0]
    out = np.zeros((C, NBUF), np.float32)
    lr = np.arange(ROWS_IN)
    gr = r0 - 1 + lr
    ok = (gr >= 0) & (gr < H)
    for i in np.nonzero(ok)[0]:
        out[:, i * RP + 1: i * RP + 1 + W] = img[:, gr[i], :]
    return out


def _prep_weights(inputs):
    w_enc, b_enc = _f32(inputs["w_enc"]), _f32(inputs["b_enc"])
    w_dec, b_dec = _f32(inputs["w_dec"]), _f32(inputs["b_dec"])
    w_agg = _f32(inputs["w_agg"])
    w_val, b_val = _f32(inputs["w_val"]), _f32(inputs["b_val"])
    w_attn, b_attn = _f32(inputs["w_attn"]), _f32(inputs["b_attn"])

    Wdec_dup = np.empty((128, 128), np.float32)
    for m in range(128):
        Wdec_dup[:, m] = w_dec[m % 64, :]
    WencB = np.zeros((128, 128), np.float32)
    WencB[:64, :64] = w_enc.T
    WencB[64:, 64:] = w_enc.T
    Wagg2 = np.zeros((128, 2), np.float32)
    Wagg2[:64, 0] = w_agg
    Wagg2[64:, 1] = w_agg
    I128 = np.eye(128, dtype=np.float32)
    # per-delta broadcast selectors (per pair layout): K=18 rows d*2+k
    Sel18 = np.zeros((18, 9, 128), np.float32)
    for di in range(9):
        Sel18[di * 2, di, :64] = 1.0
        Sel18[di * 2 + 1, di, 64:] = 1.0
    # w_attn folded into the 3x3 values conv: [d, delta, o]
    WvalT = np.zeros((128, 9, 64), np.float32)
    for di, (r, c) in enumerate(DELTAS):
        # v'[o](x) = sum_e w_attn[o, e] * w_val[e, d, r, c] * dec[d](x+delta)
        WvalT[:, di, :] = (w_attn[:, :64] @ w_val[:, :, r, c]).T
    # fold matmuls rhs per pair: [(k*64+e), o] = w_attn[o, 64*(1+2p+k)+e]
    WfoldP = np.zeros((128, 2, 64), np.float32)
    for p in range(2):
        for k in range(2):
            blk = w_attn[:, 64 * (1 + 2 * p + k): 64 * (2 + 2 * p + k)]
            WfoldP[64 * k: 64 * (k + 1), p, :] = blk.T
    ones1 = np.ones((1, 128), np.float32)
    b_out = (w_attn[:, :64] @ b_val + b_attn).reshape(1, 64)
    b_qk = np.tile(b_dec + b_enc, 2).reshape(128, 1)

    return dict(
        Wdec_dup=_bf16(Wdec_dup), WencB=_bf16(WencB), Wagg2=_bf16(Wagg2),
        I128=_bf16(I128), Sel18=_bf16(Sel18), WvalT=_bf16(WvalT),
        WfoldP=_bf16(WfoldP), ones1=_bf16(ones1), brow=_bf16(b_out),
        b_qk=_f32(b_qk),
    )


def _prep_mask(r0):
    """mask_pix [128, T*36] bf16: per pixel tile t, partition P, col di*4+k."""
    s = np.arange(T * 128)
    pos = OB + s
    lr = pos // RP - 1
    c = pos % RP - 1
    valid_px = (s < NW) & (c >= 0) & (c < W) & (lr >= 0) & (lr < ROWS_OUT)
    m = np.zeros((T * 128, 9), np.float32)
    for di, (r, cc) in enumerate(DELTAS):
        dr, dc = r - 1, cc - 1
        ok = valid_px & (r0 + lr + dr >= 0) & (r0 + lr + dr < H) \
            & (c + dc >= 0) & (c + dc < W)
        m[:, di] = ok.astype(np.float32)
    # junk pixels: keep center neighbor on so Z > 0 (no inf/NaN downstream)
    m[~valid_px, 4] = 1.0
    m2 = np.repeat(m[:, :, None], 2, axis=2)          # [T*128, 9, 2]
    m2 = m2.reshape(T, 128, 18).transpose(1, 0, 2)    # [128, T, 18]
    return _bf16(m2.reshape(128, T * 18))


def host_prep(inputs):
    wt = _prep_weights(inputs)
    ctx = _f32(inputs["contexts"])
    dec = _f32(inputs["decoded_features"])
    cores = []
    for b in range(B):
        for q in range(4):
            r0 = q * ROWS_OUT
            ctx01 = np.zeros((128, NBUF), np.float32)
            ctx01[:64] = _pad_to_buf(ctx[0, b], r0)
            ctx01[64:] = _pad_to_buf(ctx[1, b], r0)
            ctx23 = np.zeros((128, NBUF), np.float32)
            ctx23[:64] = _pad_to_buf(ctx[2, b], r0)
            ctx23[64:] = _pad_to_buf(ctx[3, b], r0)
            core = dict(
                dec=_bf16(_pad_to_buf(dec[b], r0)),
                ctx01=_bf16(ctx01), ctx23=_bf16(ctx23),
                mask_pix=_prep_mask(r0),
            )
            core.update(wt)
            cores.append(core)
    return cores


# ---------------------------------------------------------------- bass build
def build_nc():
    import concourse.bacc as bacc
    import concourse.bass as bass
    import concourse.mybir as mybir
    import concourse.tile as tile

    f32d = mybir.dt.float32
    bf16d = mybir.dt.bfloat16
    AF = mybir.ActivationFunctionType
    OP = mybir.AluOpType

    nc = bacc.Bacc("TRN2", target_bir_lowering=False, debug=False,
                   num_devices=NCORES)

    d_dec = nc.dram_tensor("dec", [128, NBUF], bf16d, kind="ExternalInput").ap()
    d_ctx = [nc.dram_tensor("ctx01", [128, NBUF], bf16d, kind="ExternalInput").ap(),
             nc.dram_tensor("ctx23", [128, NBUF], bf16d, kind="ExternalInput").ap()]
    d_mask = nc.dram_tensor("mask_pix", [128, T * 36], bf16d,
                            kind="ExternalInput").ap()
    d_Wdec = nc.dram_tensor("Wdec_dup", [128, 128], bf16d, kind="ExternalInput").ap()
    d_Wenc = nc.dram_tensor("WencB", [128, 128], bf16d, kind="ExternalInput").ap()
    d_Wagg = nc.dram_tensor("Wagg2", [128, 2], bf16d, kind="ExternalInput").ap()
    d_I128 = nc.dram_tensor("I128", [128, 128], bf16d, kind="ExternalInput").ap()
    d_Sel = nc.dram_tensor("SelAll", [36, 18 * 128], bf16d,
                           kind="ExternalInput").ap()
    d_WvalT = nc.dram_tensor("WvalT", [128, 9, 64], bf16d, kind="ExternalInput").ap()
    d_WfoldP = nc.dram_tensor("WfoldP", [128, 2, 64], bf16d,
                              kind="ExternalInput").ap()
    d_ones1 = nc.dram_tensor("ones1", [1, 128], bf16d, kind="ExternalInput").ap()
    d_brow = nc.dram_tensor("brow", [1, 64], bf16d, kind="ExternalInput").ap()
    d_bqk = nc.dram_tensor("b_qk", [128, 1], f32d, kind="ExternalInput").ap()
    d_out = nc.dram_tensor("out", [128, T * 64], bf16d, kind="ExternalOutput").ap()

    with tile.TileContext(nc) as tc:
        from contextlib import ExitStack
        stack = ExitStack()
        with tc.tile_pool(name="const", bufs=1) as const, \
             tc.tile_pool(name="big", bufs=1) as big, \
             tc.tile_pool(name="keysp", bufs=2) as keysp, \
             tc.tile_pool(name="work", bufs=6) as work, \
             tc.tile_pool(name="tanhp", bufs=16) as tanhp, \
             tc.tile_pool(name="pp", bufs=2) as pp, \
             tc.tile_pool(name="psO", bufs=1, space="PSUM") as psO:
            psS = stack.enter_context(
                tc.tile_pool(name="psS", bufs=1, space="PSUM"))
            psA = stack.enter_context(
                tc.tile_pool(name="psA", bufs=2, space="PSUM"))

            def load(pool, ap, shape, dtype, tag):
                t = pool.tile(shape, dtype, tag=tag, name=tag)
                nc.sync.dma_start(out=t, in_=ap)
                return t

            Wdec = load(const, d_Wdec, [128, 128], bf16d, "Wdec")
            Wenc = load(const, d_Wenc, [128, 128], bf16d, "Wenc")
            Wagg = load(const, d_Wagg, [128, 2], bf16d, "Wagg")
            I128 = load(const, d_I128, [128, 128], bf16d, "I128")
            SelAll = load(const, d_Sel, [36, 18, 128], bf16d, "SelAll")
            WvalT = load(const, d_WvalT, [128, 9, 64], bf16d, "WvalT")
            WfoldP = load(const, d_WfoldP, [128, 2, 64], bf16d, "WfoldP")
            ones1 = load(const, d_ones1, [1, 128], bf16d, "ones1")
            brow = load(const, d_brow, [1, 64], bf16d, "brow")
            bqk = load(const, d_bqk, [128, 1], f32d, "bqk")
            mask_s = load(const, d_mask, [128, T, 36], bf16d, "mask")
            dec_s = load(big, d_dec, [128, NBUF], bf16d, "dec")
            ctx_s = [load(big, d_ctx[0], [128, NBUF], bf16d, "ctx01"),
                     load(big, d_ctx[1], [128, NBUF], bf16d, "ctx23")]

            # ---- output accumulator psum: [128px, T*64] f32
            psO_all = psO.tile([128, T * 64], f32d, tag="o", name="psO_all")

            # ---- values conv (w_attn folded), opens every psO region
            for di in range(9):
                base = OB + DOFF[di]
                for t in range(T):
                    nc.tensor.matmul(
                        psO_all[:, t * 64:(t + 1) * 64],
                        dec_s[:, base + t * 128: base + (t + 1) * 128],
                        WvalT[:, di, :],
                        start=(di == 0), stop=False)

            # ---- queries -> q_dup [128, TIN] bf16 (+ b_qk), q_odd
            q_dup = big.tile([128, TIN], bf16d, tag="q_dup", name="q_dup")
            nc.vector.memset(q_dup[:, PW:TIN], 0.0)
            for (c0, cw) in QCH:
                psq = psA.tile([128, 512], f32d, tag="mm", name="psq")
                nc.tensor.matmul(psq[:, 0:cw], Wdec,
                                 dec_s[:, OB + c0: OB + c0 + cw],
                                 start=True, stop=True)
                nc.scalar.activation(out=q_dup[:, c0:c0 + cw],
                                     in_=psq[:, 0:cw], func=AF.Identity,
                                     bias=bqk, scale=1.0)
            q_odd = big.tile([128, TIN], bf16d, tag="q_odd", name="q_odd")
            nc.gpsimd.memset(q_odd[:, 0:2], 0.0)
            nc.gpsimd.tensor_copy(q_odd[:, 1:TIN], q_dup[:, 0:TIN - 1])

            # ---- scores psum [128, T, 9, 4] + per-pair keys/tanh
            psS_all = psS.tile([128, T, 9, 4], f32d, tag="s", name="psS_all")
            for p in range(2):
                keys_s = keysp.tile([128, NBUF], bf16d, tag="keys",
                                    name=f"keys{p}")
                for (c0, cw) in KCH:
                    psk = psA.tile([128, 512], f32d, tag="mm", name="psk")
                    nc.tensor.matmul(psk[:, 0:cw], Wenc,
                                     ctx_s[p][:, c0:c0 + cw],
                                     start=True, stop=True)
                    nc.vector.tensor_copy(keys_s[:, c0:c0 + cw], psk[:, 0:cw])
                for di in range(9):
                    base = OB + DOFF[di]
                    tin = work.tile([128, TIN], bf16d, tag="tin", name="tin")
                    # offload 2 of 9 adds per pair to the (otherwise idle)
                    # gpsimd engine
                    eng_a = nc.gpsimd if di in (0, 5) else nc.vector
                    if base % 2 == 1:
                        eng_a.tensor_add(tin, q_odd,
                                         keys_s[:, base - 1: base - 1 + TIN])
                        off = 1
                    else:
                        eng_a.tensor_add(tin, q_dup,
                                         keys_s[:, base: base + TIN])
                        off = 0
                    th = tanhp.tile([128, TIN], bf16d, tag="tanh",
                                    name=f"tanh{p}_{di}")
                    nc.scalar.activation(out=th, in_=tin, func=AF.Tanh)
                    for t in range(T):
                        nc.tensor.matmul(
                            psS_all[:, t, di, 2 * p: 2 * p + 2],
                            th[:, off + t * 128: off + t * 128 + 128],
                            Wagg, start=True, stop=True)

            # ---- softmax in pixel-major
            p_pix = big.tile([128, T, 36], bf16d, tag="p_pix", name="p_pix")
            Zbuf = big.tile([128, T, 4], f32d, tag="Z", name="Zbuf")
            rz = big.tile([128, T, 4], bf16d, tag="rz", name="rz")
            for t in range(T):
                nc.scalar.activation(
                    out=p_pix[:, t, :],
                    in_=psS_all[:, t].rearrange("p a b -> p (a b)"),
                    func=AF.Exp)
                nc.vector.tensor_mul(p_pix[:, t, :], p_pix[:, t, :],
                                     mask_s[:, t, :])
                nc.vector.tensor_reduce(
                    out=Zbuf[:, t, :],
                    in_=p_pix[:, t, :].rearrange("p (d k) -> p k d", k=4),
                    axis=mybir.AxisListType.X, op=OP.add)
            with nc.allow_low_precision(reason="softmax recip in bf16"):
                nc.vector.reciprocal(
                    out=rz.rearrange("p a b -> p (a b)"),
                    in_=Zbuf.rearrange("p a b -> p (a b)"))
            for t in range(T):
                rzv = rz[:, t, :]
                rzb = bass.AP(tensor=rzv.tensor, offset=rzv.offset,
                              ap=[list(rzv.ap[0]), [0, 9], list(rzv.ap[-1])])
                nc.vector.tensor_mul(
                    p_pix[:, t, :].rearrange("p (d k) -> p d k", k=4),
                    p_pix[:, t, :].rearrange("p (d k) -> p d k", k=4), rzb)

            # ---- transpose p to channel-major rows [36, PW]
            pT_sb = big.tile([36, PW], bf16d, tag="pT", name="pT_sb")
            stack.close()   # free psA; keep psO/psS... (psS dead now too)
            psT = stack.enter_context(
                tc.tile_pool(name="psT", bufs=2, space="PSUM"))
            psB = stack.enter_context(
                tc.tile_pool(name="psB", bufs=3, space="PSUM"))
            for batch in range((T + 3) // 4):
                n_in = min(4, T - batch * 4)
                pst = psT.tile([36, 512], bf16d, tag="t", name="psT")
                for tt in range(n_in):
                    t = batch * 4 + tt
                    nc.tensor.transpose(pst[:, tt * 128:(tt + 1) * 128],
                                        p_pix[:, t, :], I128)
                nc.vector.tensor_copy(
                    pT_sb[:, batch * 512: batch * 512 + n_in * 128],
                    pst[:, 0:n_in * 128])

            # ---- products + folds per (pair, delta)
            gi = 0
            for p in range(2):
                for di in range(9):
                    base = OB + DOFF[di]
                    g = p * 9 + di
                    prods = pp.tile([128, PW], bf16d, tag="prods",
                                    name=f"prods{p}_{di}")
                    eng = nc.vector if PROD_ENGINE[gi] == "dve" else nc.gpsimd
                    gi += 1
                    for (c0, cw) in PCH:
                        psb = psX.tile([128, 512], f32d, tag="x", name="psb")
                        nc.tensor.matmul(psb[:, 0:cw], SelAll[:, g, :],
                                         pT_sb[:, c0:c0 + cw],
                                         start=True, stop=True)
                        eng.scalar_tensor_tensor(
                            out=prods[:, c0:c0 + cw],
                            in0=psb[:, 0:cw], scalar=1.0,
                            in1=ctx_s[p][:, base + c0: base + c0 + cw],
                            op0=OP.mult, op1=OP.mult)
                    for t in range(T):
                        nc.tensor.matmul(
                            psO_all[:, t * 64:(t + 1) * 64],
                            prods[:, t * 128:(t + 1) * 128],
                            WfoldP[:, p, :], start=False, stop=False)

            # ---- bias + leaky + out
            for t in range(T):
                nc.tensor.matmul(psO_all[:, t * 64:(t + 1) * 64],
                                 ones1, brow, start=False, stop=True)
            o_f = big.tile([128, T * 64], bf16d, tag="o_f", name="o_f")
            nc.scalar.activation(out=o_f, in_=psO_all, func=AF.Identity)
            o_sb = big.tile([128, T * 64], bf16d, tag="o_sb", name="o_sb")
            nc.vector.scalar_tensor_tensor(
                out=o_sb, in0=o_f, scalar=0.2, in1=o_f,
                op0=OP.mult, op1=OP.max)
            nc.sync.dma_start(out=d_out, in_=o_sb)
            stack.close()

    nc.compile()
    return nc


def _get_nc():
    if "nc" not in _CACHE:
        _CACHE["nc"] = build_nc()
    return _CACHE["nc"]


# ---------------------------------------------------------------- entry point
def _assemble(results):
    out = np.zeros((B, E, H, W), np.float32)
    s = np.arange(NW)
    pos = OB + s
    lr = pos // RP - 1
    c = pos % RP - 1
    sel = (c >= 0) & (c < W)
    for b in range(B):
        for q in range(4):
            res = np.asarray(results[b * 4 + q]["out"]).astype(np.float32)
            res = res.reshape(128, T, 64).transpose(1, 0, 2).reshape(T * 128, 64)
            out[b, :, q * ROWS_OUT + lr[sel], c[sel]] = res[s[sel]]
    return out


def kernel(**inputs):
    from concourse import bass_utils

    cores = host_prep(inputs)
    nc = _get_nc()
    res = bass_utils.run_bass_kernel_spmd(
        nc, [dict(c) for c in cores], core_ids=list(range(NCORES)))
    return _assemble(res.results)


# revision 6
# speedup vs baseline: 1.7107x; 1.0065x over previous
# Trainium2 Bass kernel for nn_DecoderAttention (gnn_message_passing), v2.
# Self-contained: host-side prep (numpy) + bass/Tile device kernel + SPMD run.
#
# Sharding: 8 cores = batch(2) x H-quarters(4). Each core: one batch element,
# 24 output rows (+1 halo row each side), all 4 contexts.
#
# v2 design (vs v1): scores are computed into PIXEL-major layout via
# [128px, 2]-output matmuls (cheap under the cost model: matmul cost is the
# output free size), the whole softmax (exp/mask/Z/recip/normalize) runs in
# pixel-major where probabilities are per-partition scalars, then p is
# PE-transposed back to channel-major rows and broadcast to 64 channels with
# K=2 selector matmuls. w_attn is folded into the values conv and into
# per-(delta,pair) fold matmuls so everything accumulates in one PSUM tile
# per output pixel-tile. Output is written pixel-major bf16.
import numpy as np
import ml_dtypes

E, D, K, B, H, W = 64, 128, 4, 2, 96, 96
RP = 98
ROWS_OUT = 24
ROWS_IN = ROWS_OUT + 2
NBUF = 2656               # in-buffer width (26*98=2548, padded for +-99 shifts)
OB = RP + 1               # out-window base = 99
NW = ROWS_OUT * RP - 2    # 2350 real out-window pixels
T = 19                    # pixel tiles of 128 (19*128 = 2432 >= NW)
PW = T * 128              # 2432
TIN = PW + 2              # tanh tile width (even)
NCORES = 8

BF = ml_dtypes.bfloat16
DELTAS = [(r, c) for r in range(3) for c in range(3)]
DOFF = [(r - 1) * RP + (c - 1) for r, c in DELTAS]

# chunk grids
QCH = [(i * 512, min(512, TIN - i * 512)) for i in range(5)]      # q over TIN
KCH = [(i * 512, min(512, NBUF - i * 512)) for i in range(6)]     # keys
PCH = [(i * 512, min(512, PW - i * 512)) for i in range(5)]       # products

# product stt engine split: per (pair, delta) group -> 'dve' | 'pool'
# (set after measuring; pool requires gpsimd PSUM reads to work)
PROD_ENGINE = ["dve"] * 18

_CACHE = {}


def _bf16(x):
    return np.ascontiguousarray(np.asarray(x, np.float32).astype(BF))


def _f32(x):
    return np.ascontiguousarray(np.asarray(x, np.float32))


# ---------------------------------------------------------------- host prep
def _pad_to_buf(img, r0):
    """img [C, H, W] -> [C, NBUF] zero-padded halo'd row-pitch-98 buffer."""
    C = img.shape[0]
    out = np.zeros((C, NBUF), np.float32)
    lr = np.arange(ROWS_IN)
    gr = r0 - 1 + lr
    ok = (gr >= 0) & (gr < H)
    for i in np.nonzero(ok)[0]:
        out[:, i * RP + 1: i * RP + 1 + W] = img[:, gr[i], :]
    return out


def _prep_weights(inputs):
    w_enc, b_enc = _f32(inputs["w_enc"]), _f32(inputs["b_enc"])
    w_dec, b_dec = _f32(inputs["w_dec"]), _f32(inputs["b_dec"])
    w_agg = _f32(inputs["w_agg"])
    w_val, b_val = _f32(inputs["w_val"]), _f32(inputs["b_val"])
    w_attn, b_attn = _f32(inputs["w_attn"]), _f32(inputs["b_attn"])

    Wdec_dup = np.empty((128, 128), np.float32)
    for m in range(128):
        Wdec_dup[:, m] = w_dec[m % 64, :]
    WencB = np.zeros((128, 128), np.float32)
    WencB[:64, :64] = w_enc.T
    WencB[64:, 64:] = w_enc.T
    Wagg2 = np.zeros((128, 2), np.float32)
    Wagg2[:64, 0] = w_agg
    Wagg2[64:, 1] = w_agg
    I128 = np.eye(128, dtype=np.float32)
    # per-delta broadcast selectors (per pair layout): K=18 rows d*2+k
    Sel18 = np.zeros((18, 9, 128), np.float32)
    for di in range(9):
        Sel18[di * 2, di, :64] = 1.0
        Sel18[di * 2 + 1, di, 64:] = 1.0
    # w_attn folded into the 3x3 values conv: [d, delta, o]
    WvalT = np.zeros((128, 9, 64), np.float32)
    for di, (r, c) in enumerate(DELTAS):
        # v'[o](x) = sum_e w_attn[o, e] * w_val[e, d, r, c] * dec[d](x+delta)
        WvalT[:, di, :] = (w_attn[:, :64] @ w_val[:, :, r, c]).T
    # fold matmuls rhs per pair: [(k*64+e), o] = w_attn[o, 64*(1+2p+k)+e]
    WfoldP = np.zeros((128, 2, 64), np.float32)
    for p in range(2):
        for k in range(2):
            blk = w_attn[:, 64 * (1 + 2 * p + k): 64 * (2 + 2 * p + k)]
            WfoldP[64 * k: 64 * (k + 1), p, :] = blk.T
    ones1 = np.ones((1, 128), np.float32)
    b_out = (w_attn[:, :64] @ b_val + b_attn).reshape(1, 64)
    b_qk = np.tile(b_dec + b_enc, 2).reshape(128, 1)

    return dict(
        Wdec_dup=_bf16(Wdec_dup), WencB=_bf16(WencB), Wagg2=_bf16(Wagg2),
        I128=_bf16(I128), Sel18=_bf16(Sel18), WvalT=_bf16(WvalT),
        WfoldP=_bf16(WfoldP), ones1=_bf16(ones1), brow=_bf16(b_out),
        b_qk=_f32(b_qk),
    )


def _prep_mask(r0):
    """mask_pix [128, T*36] bf16: per pixel tile t, partition P, col di*4+k."""
    s = np.arange(T * 128)
    pos = OB + s
    lr = pos // RP - 1
    c = pos % RP - 1
    valid_px = (s < NW) & (c >= 0) & (c < W) & (lr >= 0) & (lr < ROWS_OUT)
    m = np.zeros((T * 128, 9), np.float32)
    for di, (r, cc) in enumerate(DELTAS):
        dr, dc = r - 1, cc - 1
        ok = valid_px & (r0 + lr + dr >= 0) & (r0 + lr + dr < H) \
            & (c + dc >= 0) & (c + dc < W)
        m[:, di] = ok.astype(np.float32)
    # junk pixels: keep center neighbor on so Z > 0 (no inf/NaN downstream)
    m[~valid_px, 4] = 1.0
    m2 = np.repeat(m[:, :, None], 2, axis=2)          # [T*128, 9, 2]
    m2 = m2.reshape(T, 128, 18).transpose(1, 0, 2)    # [128, T, 18]
    return _bf16(m2.reshape(128, T * 18))


def host_prep(inputs):
    wt = _prep_weights(inputs)
    ctx = _f32(inputs["contexts"])
    dec = _f32(inputs["decoded_features"])
    cores = []
    for b in range(B):
        for q in range(4):
            r0 = q * ROWS_OUT
            ctx01 = np.zeros((128, NBUF), np.float32)
            ctx01[:64] = _pad_to_buf(ctx[0, b], r0)
            ctx01[64:] = _pad_to_buf(ctx[1, b], r0)
            ctx23 = np.zeros((128, NBUF), np.float32)
            ctx23[:64] = _pad_to_buf(ctx[2, b], r0)
            ctx23[64:] = _pad_to_buf(ctx[3, b], r0)
            core = dict(
                dec=_bf16(_pad_to_buf(dec[b], r0)),
                ctx01=_bf16(ctx01), ctx23=_bf16(ctx23),
                mask_pix=_prep_mask(r0),
            )
            core.update(wt)
            cores.append(core)
    return cores


# ---------------------------------------------------------------- bass build
def build_nc():
    import concourse.bacc as bacc
    import concourse.bass as bass
    import concourse.mybir as mybir
    import concourse.tile as tile

    f32d = mybir.dt.float32
    bf16d = mybir.dt.bfloat16
    AF = mybir.ActivationFunctionType
    OP = mybir.AluOpType

    nc = bacc.Bacc("TRN2", target_bir_lowering=False, debug=False,
                   num_devices=NCORES)

    d_dec = nc.dram_tensor("dec", [128, NBUF], bf16d, kind="ExternalInput").ap()
    d_ctx = [nc.dram_tensor("ctx01", [128, NBUF], bf16d, kind="ExternalInput").ap(),
             nc.dram_tensor("ctx23", [128, NBUF], bf16d, kind="ExternalInput").ap()]
    d_mask = nc.dram_tensor("mask_pix", [128, T * 36], bf16d,
                            kind="ExternalInput").ap()
    d_Wdec = nc.dram_tensor("Wdec_dup", [128, 128], bf16d, kind="ExternalInput").ap()
    d_Wenc = nc.dram_tensor("WencB", [128, 128], bf16d, kind="ExternalInput").ap()
    d_Wagg = nc.dram_tensor("Wagg2", [128, 2], bf16d, kind="ExternalInput").ap()
    d_I128 = nc.dram_tensor("I128", [128, 128], bf16d, kind="ExternalInput").ap()
    d_Sel = nc.dram_tensor("SelAll", [36, 18 * 128], bf16d,
                           kind="ExternalInput").ap()
    d_WvalT = nc.dram_tensor("WvalT", [128, 9, 64], bf16d, kind="ExternalInput").ap()
    d_WfoldP = nc.dram_tensor("WfoldP", [128, 2, 64], bf16d,
                              kind="ExternalInput").ap()
    d_ones1 = nc.dram_tensor("ones1", [1, 128], bf16d, kind="ExternalInput").ap()
    d_brow = nc.dram_tensor("brow", [1, 64], bf16d, kind="ExternalInput").ap()
    d_bqk = nc.dram_tensor("b_qk", [128, 1], f32d, kind="ExternalInput").ap()
    d_out = nc.dram_tensor("out", [128, T * 64], bf16d, kind="ExternalOutput").ap()

    with tile.TileContext(nc) as tc:
        from contextlib import ExitStack
        stack = ExitStack()
        with tc.tile_pool(name="const", bufs=1) as const, \
             tc.tile_pool(name="big", bufs=1) as big, \
             tc.tile_pool(name="keysp", bufs=2) as keysp, \
             tc.tile_pool(name="work", bufs=6) as work, \
             tc.tile_pool(name="tanhp", bufs=16) as tanhp, \
             tc.tile_pool(name="pp", bufs=2) as pp, \
             tc.tile_pool(name="psO", bufs=1, space="PSUM") as psO:
            psS = stack.enter_context(
                tc.tile_pool(name="psS", bufs=1, space="PSUM"))
            psA = stack.enter_context(
                tc.tile_pool(name="psA", bufs=2, space="PSUM"))

            def load(pool, ap, shape, dtype, tag):
                t = pool.tile(shape, dtype, tag=tag, name=tag)
                nc.sync.dma_start(out=t, in_=ap)
                return t

            Wdec = load(const, d_Wdec, [128, 128], bf16d, "Wdec")
            Wenc = load(const, d_Wenc, [128, 128], bf16d, "Wenc")
            Wagg = load(const, d_Wagg, [128, 2], bf16d, "Wagg")
            I128 = load(const, d_I128, [128, 128], bf16d, "I128")
            SelAll = load(const, d_Sel, [36, 18, 128], bf16d, "SelAll")
            WvalT = load(const, d_WvalT, [128, 9, 64], bf16d, "WvalT")
            WfoldP = load(const, d_WfoldP, [128, 2, 64], bf16d, "WfoldP")
            ones1 = load(const, d_ones1, [1, 128], bf16d, "ones1")
            brow = load(const, d_brow, [1, 64], bf16d, "brow")
            bqk = load(const, d_bqk, [128, 1], f32d, "bqk")
            mask_s = load(const, d_mask, [128, T, 36], bf16d, "mask")
            dec_s = load(big, d_dec, [128, NBUF], bf16d, "dec")
            ctx_s = [load(big, d_ctx[0], [128, NBUF], bf16d, "ctx01"),
                     load(big, d_ctx[1], [128, NBUF], bf16d, "ctx23")]

            # ---- output accumulator psum: [128px, T*64] f32
            psO_all = psO.tile([128, T * 64], f32d, tag="o", name="psO_all")

            # ---- values conv (w_attn folded), opens every psO region
            for di in range(9):
                base = OB + DOFF[di]
                for t in range(T):
                    nc.tensor.matmul(
                        psO_all[:, t * 64:(t + 1) * 64],
                        dec_s[:, base + t * 128: base + (t + 1) * 128],
                        WvalT[:, di, :],
                        start=(di == 0), stop=False)

            # ---- queries -> q_dup [128, TIN] bf16 (+ b_qk), q_odd
            q_dup = big.tile([128, TIN], bf16d, tag="q_dup", name="q_dup")
            nc.vector.memset(q_dup[:, PW:TIN], 0.0)
            for (c0, cw) in QCH:
                psq = psA.tile([128, 512], f32d, tag="mm", name="psq")
                nc.tensor.matmul(psq[:, 0:cw], Wdec,
                                 dec_s[:, OB + c0: OB + c0 + cw],
                                 start=True, stop=True)
                nc.scalar.activation(out=q_dup[:, c0:c0 + cw],
                                     in_=psq[:, 0:cw], func=AF.Identity,
                                     bias=bqk, scale=1.0)
            q_odd = big.tile([128, TIN], bf16d, tag="q_odd", name="q_odd")
            nc.gpsimd.memset(q_odd[:, 0:2], 0.0)
            nc.gpsimd.tensor_copy(q_odd[:, 1:TIN], q_dup[:, 0:TIN - 1])

            # ---- scores psum [128, T, 9, 4] + per-pair keys/tanh
            psS_all = psS.tile([128, T, 9, 4], f32d, tag="s", name="psS_all")
            for p in range(2):
                keys_s = keysp.tile([128, NBUF], bf16d, tag="keys",
                                    name=f"keys{p}")
                for (c0, cw) in KCH:
                    psk = psA.tile([128, 512], f32d, tag="mm", name="psk")
                    nc.tensor.matmul(psk[:, 0:cw], Wenc,
                                     ctx_s[p][:, c0:c0 + cw],
                                     start=True, stop=True)
                    if p == 0 and c0 >= 2048:
                        nc.scalar.activation(out=keys_s[:, c0:c0 + cw],
                                             in_=psk[:, 0:cw],
                                             func=AF.Identity)
                    else:
                        nc.vector.tensor_copy(keys_s[:, c0:c0 + cw],
                                              psk[:, 0:cw])
                for di in range(9):
                    base = OB + DOFF[di]
                    tin = work.tile([128, TIN], bf16d, tag="tin", name="tin")
                    # offload 2 of 9 adds per pair to the (otherwise idle)
                    # gpsimd engine
                    eng_a = nc.gpsimd if di in (0, 5) else nc.vector
                    if base % 2 == 1:
                        eng_a.tensor_add(tin, q_odd,
                                         keys_s[:, base - 1: base - 1 + TIN])
                        off = 1
                    else:
                        eng_a.tensor_add(tin, q_dup,
                                         keys_s[:, base: base + TIN])
                        off = 0
                    th = tanhp.tile([128, TIN], bf16d, tag="tanh",
                                    name=f"tanh{p}_{di}")
                    nc.scalar.activation(out=th, in_=tin, func=AF.Tanh)
                    for t in range(T):
                        nc.tensor.matmul(
                            psS_all[:, t, di, 2 * p: 2 * p + 2],
                            th[:, off + t * 128: off + t * 128 + 128],
                            Wagg, start=True, stop=True)

            # ---- softmax in pixel-major
            p_pix = big.tile([128, T, 36], bf16d, tag="p_pix", name="p_pix")
            Zbuf = big.tile([128, T, 4], f32d, tag="Z", name="Zbuf")
            rz = big.tile([128, T, 4], bf16d, tag="rz", name="rz")
            for t in range(T):
                nc.scalar.activation(
                    out=p_pix[:, t, :],
                    in_=psS_all[:, t].rearrange("p a b -> p (a b)"),
                    func=AF.Exp)
                nc.vector.tensor_mul(p_pix[:, t, :], p_pix[:, t, :],
                                     mask_s[:, t, :])
                nc.vector.tensor_reduce(
                    out=Zbuf[:, t, :],
                    in_=p_pix[:, t, :].rearrange("p (d k) -> p k d", k=4),
                    axis=mybir.AxisListType.X, op=OP.add)
            with nc.allow_low_precision(reason="softmax recip in bf16"):
                nc.vector.reciprocal(
                    out=rz.rearrange("p a b -> p (a b)"),
                    in_=Zbuf.rearrange("p a b -> p (a b)"))
            for t in range(T):
                rzv = rz[:, t, :]
                rzb = bass.AP(tensor=rzv.tensor, offset=rzv.offset,
                              ap=[list(rzv.ap[0]), [0, 9], list(rzv.ap[-1])])
                nc.vector.tensor_mul(
                    p_pix[:, t, :].rearrange("p (d k) -> p d k", k=4),
                    p_pix[:, t, :].rearrange("p (d k) -> p d k", k=4), rzb)

            # ---- transpose p to channel-major rows [36, PW]
            pT_sb = big.tile([36, PW], bf16d, tag="pT", name="pT_sb")
            stack.close()   # free psA; keep psO/psS... (psS dead now too)
            psT = stack.enter_context(
                tc.tile_pool(name="psT", bufs=2, space="PSUM"))
            psB = stack.enter_context(
                tc.tile_pool(name="psB", bufs=3, space="PSUM"))
            for batch in range((T + 3) // 4):
                n_in = min(4, T - batch * 4)
                pst = psT.tile([36, 512], bf16d, tag="t", name="psT")
                for tt in range(n_in):
                    t = batch * 4 + tt
                    nc.tensor.transpose(pst[:, tt * 128:(tt + 1) * 128],
                                        p_pix[:, t, :], I128)
                nc.vector.tensor_copy(
                    pT_sb[:, batch * 512: batch * 512 + n_in * 128],
                    pst[:, 0:n_in * 128])

            # ---- products + folds per (pair, delta)
            gi = 0
            for p in range(2):
                for di in range(9):
                    base = OB + DOFF[di]
                    g = p * 9 + di
                    prods = pp.tile([128, PW], bf16d, tag="prods",
                                    name=f"prods{p}_{di}")
                    eng = nc.vector if PROD_ENGINE[gi] == "dve" else nc.gpsimd
                    gi += 1
                    for (c0, cw) in PCH:
                        psb = psX.tile([128, 512], f32d, tag="x", name="psb")
                        nc.tensor.matmul(psb[:, 0:cw], SelAll[:, g, :],
                                         pT_sb[:, c0:c0 + cw],
                                         start=True, stop=True)
                        eng.scalar_tensor_tensor(
                            out=prods[:, c0:c0 + cw],
                            in0=psb[:, 0:cw], scalar=1.0,
                            in1=ctx_s[p][:, base + c0: base + c0 + cw],
                            op0=OP.mult, op1=OP.mult)
                    for t in range(T):
                        nc.tensor.matmul(
                            psO_all[:, t * 64:(t + 1) * 64],
                            prods[:, t * 128:(t + 1) * 128],
                            WfoldP[:, p, :], start=False, stop=False)

            # ---- bias + leaky + out
            for t in range(T):
                nc.tensor.matmul(psO_all[:, t * 64:(t + 1) * 64],
                                 ones1, brow, start=False, stop=True)
            o_f = big.tile([128, T * 64], bf16d, tag="o_f", name="o_f")
            nc.scalar.activation(out=o_f, in_=psO_all, func=AF.Identity)
            o_sb = big.tile([128, T * 64], bf16d, tag="o_sb", name="o_sb")
            nc.vector.scalar_tensor_tensor(
                out=o_sb, in0=o_f, scalar=0.2, in1=o_f,
                op0=OP.mult, op1=OP.max)
            nc.sync.dma_start(out=d_out, in_=o_sb)
            stack.close()

    nc.compile()
    return nc


def _get_nc():
    if "nc" not in _CACHE:
        _CACHE["nc"] = build_nc()
    return _CACHE["nc"]


# ---------------------------------------------------------------- entry point
def _assemble(results):
    out = np.zeros((B, E, H, W), np.float32)
    s = np.arange(NW)
    pos = OB + s
    lr = pos // RP - 1
    c = pos % RP - 1
    sel = (c >= 0) & (c < W)
    for b in range(B):
        for q in range(4):
            res = np.asarray(results[b * 4 + q]["out"]).astype(np.float32)
            res = res.reshape(128, T, 64).transpose(1, 0, 2).reshape(T * 128, 64)
            out[b, :, q * ROWS_OUT + lr[sel], c[sel]] = res[s[sel]]
    return out


def kernel(**inputs):
    from concourse import bass_utils

    cores = host_prep(inputs)
    nc = _get_nc()
    res = bass_utils.run_bass_kernel_spmd(
        nc, [dict(c) for c in cores], core_ids=list(range(NCORES)))
    return _assemble(res.results)


# revision 7
# speedup vs baseline: 1.7154x; 1.0027x over previous
# Trainium2 Bass kernel for nn_DecoderAttention (gnn_message_passing), v2.
# Self-contained: host-side prep (numpy) + bass/Tile device kernel + SPMD run.
#
# Sharding: 8 cores = batch(2) x H-quarters(4). Each core: one batch element,
# 24 output rows (+1 halo row each side), all 4 contexts.
#
# v2 design (vs v1): scores are computed into PIXEL-major layout via
# [128px, 2]-output matmuls (cheap under the cost model: matmul cost is the
# output free size), the whole softmax (exp/mask/Z/recip/normalize) runs in
# pixel-major where probabilities are per-partition scalars, then p is
# PE-transposed back to channel-major rows and broadcast to 64 channels with
# K=2 selector matmuls. w_attn is folded into the values conv and into
# per-(delta,pair) fold matmuls so everything accumulates in one PSUM tile
# per output pixel-tile. Output is written pixel-major bf16.
import numpy as np
import ml_dtypes

E, D, K, B, H, W = 64, 128, 4, 2, 96, 96
RP = 98
ROWS_OUT = 24
ROWS_IN = ROWS_OUT + 2
NBUF = 2656               # in-buffer width (26*98=2548, padded for +-99 shifts)
OB = RP + 1               # out-window base = 99
NW = ROWS_OUT * RP - 2    # 2350 real out-window pixels
T = 19                    # pixel tiles of 128 (19*128 = 2432 >= NW)
PW = T * 128              # 2432
TIN = PW + 2              # tanh tile width (even)
NCORES = 8

BF = ml_dtypes.bfloat16
DELTAS = [(r, c) for r in range(3) for c in range(3)]
DOFF = [(r - 1) * RP + (c - 1) for r, c in DELTAS]

# chunk grids
QCH = [(i * 512, min(512, TIN - i * 512)) for i in range(5)]      # q over TIN
KCH = [(i * 512, min(512, NBUF - i * 512)) for i in range(6)]     # keys
PCH = [(i * 512, min(512, PW - i * 512)) for i in range(5)]       # products

# product stt engine split: per (pair, delta) group -> 'dve' | 'pool'
# (set after measuring; pool requires gpsimd PSUM reads to work)
PROD_ENGINE = ["dve"] * 18

_CACHE = {}


def _bf16(x):
    return np.ascontiguousarray(np.asarray(x, np.float32).astype(BF))


def _f32(x):
    return np.ascontiguousarray(np.asarray(x, np.float32))


# ---------------------------------------------------------------- host prep
def _pad_to_buf(img, r0):
    """img [C, H, W] -> [C, NBUF] zero-padded halo'd row-pitch-98 buffer."""
    C = img.shape[0]
    out = np.zeros((C, NBUF), np.float32)
    lr = np.arange(ROWS_IN)
    gr = r0 - 1 + lr
    ok = (gr >= 0) & (gr < H)
    for i in np.nonzero(ok)[0]:
        out[:, i * RP + 1: i * RP + 1 + W] = img[:, gr[i], :]
    return out


def _prep_weights(inputs):
    w_enc, b_enc = _f32(inputs["w_enc"]), _f32(inputs["b_enc"])
    w_dec, b_dec = _f32(inputs["w_dec"]), _f32(inputs["b_dec"])
    w_agg = _f32(inputs["w_agg"])
    w_val, b_val = _f32(inputs["w_val"]), _f32(inputs["b_val"])
    w_attn, b_attn = _f32(inputs["w_attn"]), _f32(inputs["b_attn"])

    Wdec_dup = np.empty((128, 128), np.float32)
    for m in range(128):
        Wdec_dup[:, m] = w_dec[m % 64, :]
    WencB = np.zeros((128, 128), np.float32)
    WencB[:64, :64] = w_enc.T
    WencB[64:, 64:] = w_enc.T
    Wagg2 = np.zeros((128, 2), np.float32)
    Wagg2[:64, 0] = w_agg
    Wagg2[64:, 1] = w_agg
    I128 = np.eye(128, dtype=np.float32)
    # per-delta broadcast selectors (per pair layout): K=18 rows d*2+k
    Sel18 = np.zeros((18, 9, 128), np.float32)
    for di in range(9):
        Sel18[di * 2, di, :64] = 1.0
        Sel18[di * 2 + 1, di, 64:] = 1.0
    # w_attn folded into the 3x3 values conv: [d, delta, o]
    WvalT = np.zeros((128, 9, 64), np.float32)
    for di, (r, c) in enumerate(DELTAS):
        # v'[o](x) = sum_e w_attn[o, e] * w_val[e, d, r, c] * dec[d](x+delta)
        WvalT[:, di, :] = (w_attn[:, :64] @ w_val[:, :, r, c]).T
    # fold matmuls rhs per pair: [(k*64+e), o] = w_attn[o, 64*(1+2p+k)+e]
    WfoldP = np.zeros((128, 2, 64), np.float32)
    for p in range(2):
        for k in range(2):
            blk = w_attn[:, 64 * (1 + 2 * p + k): 64 * (2 + 2 * p + k)]
            WfoldP[64 * k: 64 * (k + 1), p, :] = blk.T
    ones1 = np.ones((1, 128), np.float32)
    b_out = (w_attn[:, :64] @ b_val + b_attn).reshape(1, 64)
    b_qk = np.tile(b_dec + b_enc, 2).reshape(128, 1)

    return dict(
        Wdec_dup=_bf16(Wdec_dup), WencB=_bf16(WencB), Wagg2=_bf16(Wagg2),
        I128=_bf16(I128), Sel18=_bf16(Sel18), WvalT=_bf16(WvalT),
        WfoldP=_bf16(WfoldP), ones1=_bf16(ones1), brow=_bf16(b_out),
        b_qk=_f32(b_qk),
    )


def _prep_mask(r0):
    """mask_pix [128, T*36] bf16: per pixel tile t, partition P, col di*4+k."""
    s = np.arange(T * 128)
    pos = OB + s
    lr = pos // RP - 1
    c = pos % RP - 1
    valid_px = (s < NW) & (c >= 0) & (c < W) & (lr >= 0) & (lr < ROWS_OUT)
    m = np.zeros((T * 128, 9), np.float32)
    for di, (r, cc) in enumerate(DELTAS):
        dr, dc = r - 1, cc - 1
        ok = valid_px & (r0 + lr + dr >= 0) & (r0 + lr + dr < H) \
            & (c + dc >= 0) & (c + dc < W)
        m[:, di] = ok.astype(np.float32)
    # junk pixels: keep center neighbor on so Z > 0 (no inf/NaN downstream)
    m[~valid_px, 4] = 1.0
    m2 = np.repeat(m[:, :, None], 2, axis=2)          # [T*128, 9, 2]
    m2 = m2.reshape(T, 128, 18).transpose(1, 0, 2)    # [128, T, 18]
    return _bf16(m2.reshape(128, T * 18))


def host_prep(inputs):
    wt = _prep_weights(inputs)
    ctx = _f32(inputs["contexts"])
    dec = _f32(inputs["decoded_features"])
    cores = []
    for b in range(B):
        for q in range(4):
            r0 = q * ROWS_OUT
            ctx01 = np.zeros((128, NBUF), np.float32)
            ctx01[:64] = _pad_to_buf(ctx[0, b], r0)
            ctx01[64:] = _pad_to_buf(ctx[1, b], r0)
            ctx23 = np.zeros((128, NBUF), np.float32)
            ctx23[:64] = _pad_to_buf(ctx[2, b], r0)
            ctx23[64:] = _pad_to_buf(ctx[3, b], r0)
            core = dict(
                dec=_bf16(_pad_to_buf(dec[b], r0)),
                ctx01=_bf16(ctx01), ctx23=_bf16(ctx23),
                mask_pix=_prep_mask(r0),
            )
            core.update(wt)
            cores.append(core)
    return cores


# ---------------------------------------------------------------- bass build
def build_nc():
    import concourse.bacc as bacc
    import concourse.bass as bass
    import concourse.mybir as mybir
    import concourse.tile as tile

    f32d = mybir.dt.float32
    bf16d = mybir.dt.bfloat16
    AF = mybir.ActivationFunctionType
    OP = mybir.AluOpType

    nc = bacc.Bacc("TRN2", target_bir_lowering=False, debug=False,
                   num_devices=NCORES)

    d_dec = nc.dram_tensor("dec", [128, NBUF], bf16d, kind="ExternalInput").ap()
    d_ctx = [nc.dram_tensor("ctx01", [128, NBUF], bf16d, kind="ExternalInput").ap(),
             nc.dram_tensor("ctx23", [128, NBUF], bf16d, kind="ExternalInput").ap()]
    d_mask = nc.dram_tensor("mask_pix", [128, T * 36], bf16d,
                            kind="ExternalInput").ap()
    d_Wdec = nc.dram_tensor("Wdec_dup", [128, 128], bf16d, kind="ExternalInput").ap()
    d_Wenc = nc.dram_tensor("WencB", [128, 128], bf16d, kind="ExternalInput").ap()
    d_Wagg = nc.dram_tensor("Wagg2", [128, 2], bf16d, kind="ExternalInput").ap()
    d_I128 = nc.dram_tensor("I128", [128, 128], bf16d, kind="ExternalInput").ap()
    d_Sel = nc.dram_tensor("SelAll", [36, 18 * 128], bf16d,
                           kind="ExternalInput").ap()
    d_WvalT = nc.dram_tensor("WvalT", [128, 9, 64], bf16d, kind="ExternalInput").ap()
    d_WfoldP = nc.dram_tensor("WfoldP", [128, 2, 64], bf16d,
                              kind="ExternalInput").ap()
    d_ones1 = nc.dram_tensor("ones1", [1, 128], bf16d, kind="ExternalInput").ap()
    d_brow = nc.dram_tensor("brow", [1, 64], bf16d, kind="ExternalInput").ap()
    d_bqk = nc.dram_tensor("b_qk", [128, 1], f32d, kind="ExternalInput").ap()
    d_out = nc.dram_tensor("out", [128, T * 64], bf16d, kind="ExternalOutput").ap()

    with tile.TileContext(nc) as tc:
        from contextlib import ExitStack
        stack = ExitStack()
        with tc.tile_pool(name="const", bufs=1) as const, \
             tc.tile_pool(name="big", bufs=1) as big, \
             tc.tile_pool(name="keysp", bufs=2) as keysp, \
             tc.tile_pool(name="work", bufs=6) as work, \
             tc.tile_pool(name="tanhp", bufs=16) as tanhp, \
             tc.tile_pool(name="pp", bufs=2) as pp, \
             tc.tile_pool(name="psO", bufs=1, space="PSUM") as psO:
            psS = stack.enter_context(
                tc.tile_pool(name="psS", bufs=1, space="PSUM"))
            psA = stack.enter_context(
                tc.tile_pool(name="psA", bufs=2, space="PSUM"))

            def load(pool, ap, shape, dtype, tag):
                t = pool.tile(shape, dtype, tag=tag, name=tag)
                nc.sync.dma_start(out=t, in_=ap)
                return t

            Wdec = load(const, d_Wdec, [128, 128], bf16d, "Wdec")
            Wenc = load(const, d_Wenc, [128, 128], bf16d, "Wenc")
            Wagg = load(const, d_Wagg, [128, 2], bf16d, "Wagg")
            I128 = load(const, d_I128, [128, 128], bf16d, "I128")
            SelAll = load(const, d_Sel, [36, 18, 128], bf16d, "SelAll")
            WvalT = load(const, d_WvalT, [128, 9, 64], bf16d, "WvalT")
            WfoldP = load(const, d_WfoldP, [128, 2, 64], bf16d, "WfoldP")
            ones1 = load(const, d_ones1, [1, 128], bf16d, "ones1")
            brow = load(const, d_brow, [1, 64], bf16d, "brow")
            bqk = load(const, d_bqk, [128, 1], f32d, "bqk")
            mask_s = load(const, d_mask, [128, T, 36], bf16d, "mask")
            dec_s = load(big, d_dec, [128, NBUF], bf16d, "dec")
            ctx_s = [load(big, d_ctx[0], [128, NBUF], bf16d, "ctx01"),
                     load(big, d_ctx[1], [128, NBUF], bf16d, "ctx23")]

            # ---- output accumulator psum: [128px, T*64] f32
            psO_all = psO.tile([128, T * 64], f32d, tag="o", name="psO_all")

            # ---- values conv (w_attn folded), opens every psO region
            for di in range(9):
                base = OB + DOFF[di]
                for t in range(T):
                    nc.tensor.matmul(
                        psO_all[:, t * 64:(t + 1) * 64],
                        dec_s[:, base + t * 128: base + (t + 1) * 128],
                        WvalT[:, di, :],
                        start=(di == 0), stop=False)

            # ---- queries -> q_dup [128, TIN] bf16 (+ b_qk), q_odd
            q_dup = big.tile([128, TIN], bf16d, tag="q_dup", name="q_dup")
            nc.vector.memset(q_dup[:, PW:TIN], 0.0)
            for (c0, cw) in QCH:
                psq = psA.tile([128, 512], f32d, tag="mm", name="psq")
                nc.tensor.matmul(psq[:, 0:cw], Wdec,
                                 dec_s[:, OB + c0: OB + c0 + cw],
                                 start=True, stop=True)
                if c0 >= 2048:
                    nc.vector.tensor_scalar(
                        out=q_dup[:, c0:c0 + cw], in0=psq[:, 0:cw],
                        scalar1=bqk[:, 0:1], scalar2=None,
                        op0=OP.add)
                else:
                    nc.scalar.activation(out=q_dup[:, c0:c0 + cw],
                                         in_=psq[:, 0:cw], func=AF.Identity,
                                         bias=bqk, scale=1.0)
            q_odd = big.tile([128, TIN], bf16d, tag="q_odd", name="q_odd")
            nc.gpsimd.memset(q_odd[:, 0:2], 0.0)
            nc.gpsimd.tensor_copy(q_odd[:, 1:TIN], q_dup[:, 0:TIN - 1])

            # ---- scores psum [128, T, 9, 4] + per-pair keys/tanh
            psS_all = psS.tile([128, T, 9, 4], f32d, tag="s", name="psS_all")
            for p in range(2):
                keys_s = keysp.tile([128, NBUF], bf16d, tag="keys",
                                    name=f"keys{p}")
                for (c0, cw) in KCH:
                    psk = psA.tile([128, 512], f32d, tag="mm", name="psk")
                    nc.tensor.matmul(psk[:, 0:cw], Wenc,
                                     ctx_s[p][:, c0:c0 + cw],
                                     start=True, stop=True)
                    if p == 0 and c0 >= 2048:
                        nc.scalar.activation(out=keys_s[:, c0:c0 + cw],
                                             in_=psk[:, 0:cw],
                                             func=AF.Identity)
                    else:
                        nc.vector.tensor_copy(keys_s[:, c0:c0 + cw],
                                              psk[:, 0:cw])
                for di in range(9):
                    base = OB + DOFF[di]
                    tin = work.tile([128, TIN], bf16d, tag="tin", name="tin")
                    # offload 2 of 9 adds per pair to the (otherwise idle)
                    # gpsimd engine
                    eng_a = nc.gpsimd if di in (0, 5) else nc.vector
                    if base % 2 == 1:
                        eng_a.tensor_add(tin, q_odd,
                                         keys_s[:, base - 1: base - 1 + TIN])
                        off = 1
                    else:
                        eng_a.tensor_add(tin, q_dup,
                                         keys_s[:, base: base + TIN])
                        off = 0
                    th = tanhp.tile([128, TIN], bf16d, tag="tanh",
                                    name=f"tanh{p}_{di}")
                    nc.scalar.activation(out=th, in_=tin, func=AF.Tanh)
                    for t in range(T):
                        nc.tensor.matmul(
                            psS_all[:, t, di, 2 * p: 2 * p + 2],
                            th[:, off + t * 128: off + t * 128 + 128],
                            Wagg, start=True, stop=True)

            # ---- softmax in pixel-major
            p_pix = big.tile([128, T, 36], bf16d, tag="p_pix", name="p_pix")
            Zbuf = big.tile([128, T, 4], f32d, tag="Z", name="Zbuf")
            rz = big.tile([128, T, 4], bf16d, tag="rz", name="rz")
            for t in range(T):
                nc.scalar.activation(
                    out=p_pix[:, t, :],
                    in_=psS_all[:, t].rearrange("p a b -> p (a b)"),
                    func=AF.Exp)
                nc.vector.tensor_mul(p_pix[:, t, :], p_pix[:, t, :],
                                     mask_s[:, t, :])
                nc.vector.tensor_reduce(
                    out=Zbuf[:, t, :],
                    in_=p_pix[:, t, :].rearrange("p (d k) -> p k d", k=4),
                    axis=mybir.AxisListType.X, op=OP.add)
            with nc.allow_low_precision(reason="softmax recip in bf16"):
                nc.vector.reciprocal(
                    out=rz.rearrange("p a b -> p (a b)"),
                    in_=Zbuf.rearrange("p a b -> p (a b)"))
            for t in range(T):
                rzv = rz[:, t, :]
                rzb = bass.AP(tensor=rzv.tensor, offset=rzv.offset,
                              ap=[list(rzv.ap[0]), [0, 9], list(rzv.ap[-1])])
                nc.vector.tensor_mul(
                    p_pix[:, t, :].rearrange("p (d k) -> p d k", k=4),
                    p_pix[:, t, :].rearrange("p (d k) -> p d k", k=4), rzb)

            # ---- transpose p to channel-major rows [36, PW]
            pT_sb = big.tile([36, PW], bf16d, tag="pT", name="pT_sb")
            stack.close()   # free psA; keep psO/psS... (psS dead now too)
            psT = stack.enter_context(
                tc.tile_pool(name="psT", bufs=2, space="PSUM"))
            psB = stack.enter_context(
                tc.tile_pool(name="psB", bufs=3, space="PSUM"))
            for batch in range((T + 3) // 4):
                n_in = min(4, T - batch * 4)
                pst = psT.tile([36, 512], bf16d, tag="t", name="psT")
                for tt in range(n_in):
                    t = batch * 4 + tt
                    nc.tensor.transpose(pst[:, tt * 128:(tt + 1) * 128],
                                        p_pix[:, t, :], I128)
                nc.vector.tensor_copy(
                    pT_sb[:, batch * 512: batch * 512 + n_in * 128],
                    pst[:, 0:n_in * 128])

            # ---- products + folds per (pair, delta)
            gi = 0
            for p in range(2):
                for di in range(9):
                    base = OB + DOFF[di]
                    g = p * 9 + di
                    prods = pp.tile([128, PW], bf16d, tag="prods",
                                    name=f"prods{p}_{di}")
                    eng = nc.vector if PROD_ENGINE[gi] == "dve" else nc.gpsimd
                    gi += 1
                    for (c0, cw) in PCH:
                        psb = psX.tile([128, 512], f32d, tag="x", name="psb")
                        nc.tensor.matmul(psb[:, 0:cw], SelAll[:, g, :],
                                         pT_sb[:, c0:c0 + cw],
                                         start=True, stop=True)
                        eng.scalar_tensor_tensor(
                            out=prods[:, c0:c0 + cw],
                            in0=psb[:, 0:cw], scalar=1.0,
                            in1=ctx_s[p][:, base + c0: base + c0 + cw],
                            op0=OP.mult, op1=OP.mult)
                    for t in range(T):
                        nc.tensor.matmul(
                            psO_all[:, t * 64:(t + 1) * 64],
                            prods[:, t * 128:(t + 1) * 128],
                            WfoldP[:, p, :], start=False, stop=False)

            # ---- bias + leaky + out
            for t in range(T):
                nc.tensor.matmul(psO_all[:, t * 64:(t + 1) * 64],
                                 ones1, brow, start=False, stop=True)
            o_f = big.tile([128, T * 64], bf16d, tag="o_f", name="o_f")
            nc.scalar.activation(out=o_f, in_=psO_all, func=AF.Identity)
            o_sb = big.tile([128, T * 64], bf16d, tag="o_sb", name="o_sb")
            nc.vector.scalar_tensor_tensor(
                out=o_sb, in0=o_f, scalar=0.2, in1=o_f,
                op0=OP.mult, op1=OP.max)
            nc.sync.dma_start(out=d_out, in_=o_sb)
            stack.close()

    nc.compile()
    return nc


def _get_nc():
    if "nc" not in _CACHE:
        _CACHE["nc"] = build_nc()
    return _CACHE["nc"]


# ---------------------------------------------------------------- entry point
def _assemble(results):
    out = np.zeros((B, E, H, W), np.float32)
    s = np.arange(NW)
    pos = OB + s
    lr = pos // RP - 1
    c = pos % RP - 1
    sel = (c >= 0) & (c < W)
    for b in range(B):
        for q in range(4):
            res = np.asarray(results[b * 4 + q]["out"]).astype(np.float32)
            res = res.reshape(128, T, 64).transpose(1, 0, 2).reshape(T * 128, 64)
            out[b, :, q * ROWS_OUT + lr[sel], c[sel]] = res[s[sel]]
    return out


def kernel(**inputs):
    from concourse import bass_utils

    cores = host_prep(inputs)
    nc = _get_nc()
    res = bass_utils.run_bass_kernel_spmd(
        nc, [dict(c) for c in cores], core_ids=list(range(NCORES)))
    return _assemble(res.results)
